# revision 32
# baseline (speedup 1.0000x reference)
"""Trainium2 Bass kernel for a 2-layer GATv2 aggregator (N=50000, E=800000).

Self-contained: kernel(**inputs) takes full inputs, shards across 8
NeuronCores internally, returns the full (50000, 128) float32 output.

v2 strategy (8-core SPMD, dst-sharded):
- Channels permuted per head (positive-att first) and tables pre-scaled by the
  SIGNED att value: t~_c = att_c*(xl_c + xr_c).  Then
  att_c*leaky(t_c) = Prelu(t~_c; 0.2) for att_c>0 and min(t~, 0.2 t~)
  = Prelu(0.2*t~; 5) for att_c<0, so alpha = plain per-head sum of the
  ACT output -- one strided reduce, no per-edge att multiply.
- Messages aggregate Sum p*x~l (scaled); epilogue divides by att_c per
  channel (recip const tile).  Layer-2 weight rows pre-permuted; final
  output unpermuted on host.
- All edge-pass tiles bf16 (tables, one-hots, rhs);  eps term
  1e-16*exp(sum alpha) (replicates the oracle's segment_max-is-sum bug)
  accumulated via exact hi/lo bf16 split columns in the scatter matmul.
- dma_gather with prepare_only+trigger_dma so SWDGE desc-gen overlaps
  the DMA drain;  gather calls of 2048 edges (bf16 rows, 256 B).
"""
import numpy as np
import ml_dtypes

import concourse.bass as bass
import concourse.bacc as bacc
import concourse.mybir as mybir
from concourse.tile import TileContext

BF16 = ml_dtypes.bfloat16
F32 = mybir.dt.float32
BF = mybir.dt.bfloat16
I16 = mybir.dt.int16
PAD_DST = 200.0
P = 128
CPC = 8           # chunks per compute group
GPC = 8           # chunks per gather call (1024 idx = SWDGE ring max)
NI = GPC * 128    # indices per gather call
import os
USE_PREP = os.environ.get("GAT_PREP", "0") == "1"


class Cfg:
    def __init__(self, N, E, nblk, feat=128, heads1=2):
        self.N, self.E = N, E
        self.NBLK = nblk
        self.SHARD = nblk * P
        self.NPAD = 8 * self.SHARD
        assert self.NPAD >= N and self.NPAD % 256 == 0
        self.HALF = self.NPAD // 2
        assert self.HALF <= 32767
        self.F = feat
        self.H1 = heads1
        self.C1 = feat // heads1


def host_prep(cfg, x, edge_index):
    """Returns (per_core_inputs: list of dict, struct: dict)."""
    N, E = cfg.N, cfg.E
    src = np.concatenate([np.asarray(edge_index[0]), np.arange(N)]).astype(np.int64)
    dst = np.concatenate([np.asarray(edge_index[1]), np.arange(N)]).astype(np.int64)
    ET = src.shape[0]

    core = dst // cfg.SHARD
    block = (dst % cfg.SHARD) // P
    dloc = dst % P
    half = (src >= cfg.HALF).astype(np.int64)
    gval = (src - half * cfg.HALF).astype(np.int64)

    # group = (core, half, block); rank within group
    key = (core * 2 + half) * cfg.NBLK + block
    order = np.argsort(key, kind="stable")
    key_s = key[order]
    ngroups = 8 * 2 * cfg.NBLK
    cnt = np.bincount(key_s, minlength=ngroups)
    starts = np.zeros(ngroups + 1, np.int64)
    np.cumsum(cnt, out=starts[1:])
    rank = np.arange(ET) - starts[key_s]

    cnt3 = cnt.reshape(8, 2, cfg.NBLK)
    S_A = int(np.ceil(cnt3[:, 0, :].max() / P))
    S_B = int(np.ceil(cnt3[:, 1, :].max() / P))
    S_A, S_B = max(S_A, 1), max(S_B, 1)
    CHA = -(-(cfg.NBLK * S_A) // GPC) * GPC
    CHB = -(-(cfg.NBLK * S_B) // GPC) * GPC
    CH = CHA + CHB
    CALLS = CH // GPC

    chunk_half = np.zeros(CH, np.int64)
    chunk_block = np.zeros(CH, np.int64)
    for c in range(CH):
        if c < CHA:
            chunk_half[c] = 0
            chunk_block[c] = min(c // S_A, cfg.NBLK - 1)
        else:
            chunk_half[c] = 1
            chunk_block[c] = min((c - CHA) // S_B, cfg.NBLK - 1)

    gidx = np.zeros((8, CH, P), np.int16)
    dstl = np.full((8, CH, P), PAD_DST, np.float32)
    g_half = half[order]
    g_core = core[order]
    g_block = block[order]
    slot_base = np.where(g_half == 0, g_block * S_A, CHA + g_block * S_B)
    slot = slot_base + rank // P
    pos = rank % P
    gidx[g_core, slot, pos] = gval[order].astype(np.int16)
    dstl[g_core, slot, pos] = dloc[order].astype(np.float32)

    # host-built scatter one-hots qt[e, n] = (dstl == n), streamed to SBUF by
    # HWDGE DMA (keeps the build off DVE, which serializes with SWDGE gathers)
    qts = np.zeros((8, P, CH * P), BF16)
    narr = np.arange(P, dtype=np.float32)
    for k in range(8):
        oh = (dstl[k][:, :, None] == narr[None, None, :])  # [CH, e, n]
        qts[k] = np.ascontiguousarray(
            oh.transpose(1, 0, 2).reshape(P, CH * P)).astype(BF16)

    # wrap gather indices per call of NI: [NI//16,16].T -> [16, NI//16]
    gw = gidx.reshape(8, CALLS, NI // 16, 16).transpose(0, 1, 3, 2)
    gw = gw.transpose(0, 2, 1, 3).reshape(8, 16, CALLS * (NI // 16))
    gw = np.tile(gw, (1, 8, 1))  # replicate to 128 partitions

    struct = dict(S_A=S_A, S_B=S_B, CHA=CHA, CHB=CHB, CH=CH, CALLS=CALLS,
                  chunk_half=chunk_half, chunk_block=chunk_block)

    x_pad = np.zeros((cfg.NPAD, cfg.F), np.float32)
    x_pad[:N] = np.asarray(x, np.float32)

    per_core = []
    for k in range(8):
        per_core.append(dict(
            xTs=np.ascontiguousarray(
                x_pad[k * cfg.SHARD:(k + 1) * cfg.SHARD].T.astype(BF16)),
            gidx=np.ascontiguousarray(gw[k]),
            qts=qts[k],
        ))
    return per_core, struct


def _perm_layer(Wl, Wr, att):
    """Channel perm (positive att first per head) + signed-scale weights."""
    att = np.asarray(att, np.float32)
    H, C = att.shape
    perm = np.zeros((H, C), np.int64)
    k = np.zeros(H, np.int64)
    for h in range(H):
        pos = np.where(att[h] > 0)[0]
        neg = np.where(att[h] <= 0)[0]
        perm[h] = np.concatenate([pos, neg])
        k[h] = len(pos)
    att_p = np.take_along_axis(att, perm, axis=1)
    s = att_p.reshape(-1)                    # signed scale per (permuted) chan
    flat_perm = (perm + np.arange(H)[:, None] * C).reshape(-1)
    Wl_t = np.asarray(Wl, np.float32)[:, flat_perm] * s[None, :]
    Wr_t = np.asarray(Wr, np.float32)[:, flat_perm] * s[None, :]
    return Wl_t, Wr_t, s, k, flat_perm


def host_consts(cfg, Wl1, Wr1, att1, b1, Wl2, Wr2, att2, b2):
    f = cfg.F
    Wl1t, Wr1t, s1, k1, perm1 = _perm_layer(Wl1, Wr1, att1)
    # layer2 rows permuted by perm1 (its input h is in permuted-1 order)
    Wl2t, Wr2t, s2, k2, perm2 = _perm_layer(
        np.asarray(Wl2, np.float32)[perm1], np.asarray(Wr2, np.float32)[perm1],
        att2)
    c = {}
    c["w1"] = np.hstack([Wl1t, Wr1t]).astype(BF16)
    c["w2"] = np.hstack([Wl2t, Wr2t]).astype(BF16)
    c["recip1"] = np.tile((1.0 / s1).reshape(1, f), (P, 1)).astype(np.float32)
    c["recip2"] = np.tile((1.0 / s2).reshape(1, f), (P, 1)).astype(np.float32)
    c["bb1"] = np.tile(np.asarray(b1, np.float32)[perm1].reshape(1, f), (P, 1))
    c["bb2"] = np.tile(np.asarray(b2, np.float32)[perm2].reshape(1, f), (P, 1))
    c["identB"] = np.eye(P, dtype=np.float32).astype(BF16)
    c["identF"] = np.eye(P, dtype=np.float32)
    meta = dict(k1=tuple(int(v) for v in k1), k2=tuple(int(v) for v in k2),
                perm2=perm2)
    return c, meta


def _ap(base, layout, extra_offset=0):
    return bass.AP(base.tensor, base.offset + extra_offset,
                   [list(d) for d in layout])


def build_program(cfg, struct, k1, k2):
    NBLK, SHARD, NPAD, HALF, F = cfg.NBLK, cfg.SHARD, cfg.NPAD, cfg.HALF, cfg.F
    CH, CALLS = struct["CH"], struct["CALLS"]
    chunk_half, chunk_block = struct["chunk_half"], struct["chunk_block"]
    RW = F + 8

    nc = bacc.Bacc("TRN2", target_bir_lowering=False, debug=False,
                   num_devices=8, num_swdge_queues=4)

    xTs = nc.dram_tensor("xTs", [P, SHARD], BF, kind="ExternalInput")
    gidx = nc.dram_tensor("gidx", [P, CALLS * (NI // 16)], I16, kind="ExternalInput")
    qts = nc.dram_tensor("qts", [P, CH * P], BF, kind="ExternalInput")
    w1 = nc.dram_tensor("w1", [P, 2 * F], BF, kind="ExternalInput")
    w2 = nc.dram_tensor("w2", [P, 2 * F], BF, kind="ExternalInput")
    recip1 = nc.dram_tensor("recip1", [P, F], F32, kind="ExternalInput")
    recip2 = nc.dram_tensor("recip2", [P, F], F32, kind="ExternalInput")
    bb1 = nc.dram_tensor("bb1", [P, F], F32, kind="ExternalInput")
    bb2 = nc.dram_tensor("bb2", [P, F], F32, kind="ExternalInput")
    identB = nc.dram_tensor("identB", [P, P], BF, kind="ExternalInput")
    identF = nc.dram_tensor("identF", [P, P], F32, kind="ExternalInput")
    out = nc.dram_tensor("out", [SHARD, F], F32, kind="ExternalOutput")

    eq = mybir.AluOpType.is_equal
    mul = mybir.AluOpType.mult
    AF = mybir.ActivationFunctionType
    AX = mybir.AxisListType.X

    with TileContext(nc) as tc:
        with (
            tc.tile_pool(name="const", bufs=1) as cpool,
            tc.tile_pool(name="big", bufs=1) as bigp,
            tc.tile_pool(name="work", bufs=1) as wp,
            tc.tile_pool(name="psum", bufs=1, space="PSUM") as pp,
            tc.tile_pool(name="dram", bufs=1, space="DRAM") as dp,
        ):
            def load_const(t, shape, dt):
                s = cpool.tile(shape, dt, name=t.name + "_sb")
                nc.sync.dma_start(out=s[:], in_=t[:])
                return s
            w1_sb = load_const(w1, [P, 2 * F], BF)
            w2_sb = load_const(w2, [P, 2 * F], BF)
            recip1_sb = load_const(recip1, [P, F], F32)
            recip2_sb = load_const(recip2, [P, F], F32)
            bb1_sb = load_const(bb1, [P, F], F32)
            bb2_sb = load_const(bb2, [P, F], F32)
            identB_sb = load_const(identB, [P, P], BF)
            identF_sb = load_const(identF, [P, P], F32)
            xTs_sb = bigp.tile([P, SHARD], BF, name="xTs_sb")
            nc.sync.dma_start(out=xTs_sb[:], in_=xTs[:])
            gidx_sb = bigp.tile([P, CALLS * (NI // 16)], I16, name="gidx_sb")
            nc.sync.dma_start(out=gidx_sb[:], in_=gidx[:])

            lneps_sb = cpool.tile([P, 1], F32, name="lneps_sb")
            nc.vector.memset(lneps_sb[:], float(np.log(1e-16)))
            xr1_sb = bigp.tile([P, SHARD], BF, name="xr1_sb")
            xr2_sb = bigp.tile([P, SHARD], BF, name="xr2_sb")
            hT_sb = bigp.tile([P, SHARD], BF, name="hT_sb")
            hacc = bigp.tile([P, NBLK * RW], F32, name="hacc")
            stage = bigp.tile([P, SHARD], BF, name="stage")      # xl (bf16)
            stage_o = bigp.tile([P, SHARD], F32, name="stage_o")  # epilogue f32

            xl1sh = dp.tile([SHARD, F], BF, name="xl1sh")
            xl1full = dp.tile([NPAD, F], BF, name="xl1full", addr_space="Shared")
            xl2sh = dp.tile([SHARD, F], BF, name="xl2sh")
            xl2full = dp.tile([NPAD, F], BF, name="xl2full", addr_space="Shared")

            dma_sem = nc.alloc_semaphore("gat_dma")

            def node_phase(src_sb, w_sb, xr_dst):
                for j in range(NBLK):
                    mm = pp.tile([P, 8, P], F32, tag="txr", bufs=2, name=f"mm{j}")
                    mf = mm[:].rearrange("p c f -> p (c f)")
                    nc.tensor.matmul(out=mf[:, 0:2 * F],
                                     lhsT=src_sb[:, j * P:(j + 1) * P],
                                     rhs=w_sb[:], start=True, stop=True)
                    nc.scalar.activation(out=stage[:, j * F:(j + 1) * F],
                                         in_=mf[:, 0:F], func=AF.Copy)
                    nc.scalar.activation(out=xr_dst[:, j * P:(j + 1) * P],
                                         in_=mf[:, F:2 * F], func=AF.Copy)

            def dma_stage_to(dram_tile):
                o = dram_tile[:].rearrange("(b p) f -> p b f", p=P)
                i = stage[:].rearrange("p (b f) -> p b f", f=F)
                nc.sync.dma_start(out=o, in_=i)

            def edge_pass(layer, table, xr_sb, ks):
                HN = cfg.H1 if layer == 1 else 1
                CW = F // HN
                RWB = F + 3 * HN
                nc.vector.memset(hacc[:], 0.0)
                bp = None
                for g in range(CALLS):
                    cb0 = g * GPC
                    hf = int(chunk_half[cb0])
                    tab = table[:][0:HALF, :] if hf == 0 else table[:][HALF:NPAD, :]
                    xg = wp.tile([P, GPC, F], BF, tag="xg", bufs=3,
                                 name=f"xg{layer}_{g}")
                    if USE_PREP:
                        nc.gpsimd.dma_gather(
                            out_ap=xg[:], in_ap=tab,
                            idxs_ap=gidx_sb[:, g * (NI // 16):(g + 1) * (NI // 16)],
                            num_idxs=NI, num_idxs_reg=NI, elem_size=F,
                            prepare_only=True, sem=dma_sem, queue_num=0)
                        nc.gpsimd.trigger_dma(count=None)
                    else:
                        nc.gpsimd.dma_gather(
                            out_ap=xg[:], in_ap=tab,
                            idxs_ap=gidx_sb[:, g * (NI // 16):(g + 1) * (NI // 16)],
                            num_idxs=NI, num_idxs_reg=NI, elem_size=F,
                            queue_num=g % 4)
                    for sub in range(GPC // CPC):
                        cb = cb0 + sub * CPC
                        # one-hot QT[e, n] (lhsT for scatter) streamed from host
                        qt = wp.tile([P, CPC, P], BF, tag="qt", bufs=3,
                                     name=f"qt{layer}_{cb}")
                        nc.sync.dma_start(
                            out=qt[:],
                            in_=qts[:, cb * P:(cb + CPC) * P].rearrange(
                                "p (c f) -> p c f", f=P))
                        # Q[n, e] one-hot = PE transpose of QT; copy to SBUF on ACT
                        trp = pp.tile([P, CPC * P], BF, tag="trp", bufs=2,
                                      name=f"trp{layer}_{cb}")
                        for c in range(CPC):
                            nc.tensor.transpose(
                                out=trp[:, c * P:(c + 1) * P],
                                in_=qt[:, c, :], identity=identB_sb[:])
                        q = wp.tile([P, CPC, P], BF, tag="q", bufs=3,
                                    name=f"q{layer}_{cb}")
                        nc.scalar.activation(
                            out=q[:], in_=trp[:].rearrange("p (c f) -> p c f", f=P),
                            func=AF.Copy)
                        # t~ = Q.T @ xr (+ xg via identity matmul), PSUM f32
                        txr = pp.tile([P, CPC, P], F32, tag="txr", bufs=2,
                                      name=f"txr{layer}_{cb}")
                        for c in range(CPC):
                            blk = int(chunk_block[cb + c])
                            nc.tensor.matmul(
                                out=txr[:, c, :], lhsT=q[:, c, :],
                                rhs=xr_sb[:, blk * P:(blk + 1) * P],
                                start=True, stop=False)
                            nc.tensor.matmul(
                                out=txr[:, c, :], lhsT=identB_sb[:],
                                rhs=xg[:, sub * CPC + c, :],
                                start=False, stop=True)
                        # att_c*leaky(t_c): Prelu(x;.2) pos block,
                        # Prelu(.2x;5)=min(x,.2x) neg block, per head
                        lr = wp.tile([P, CPC, F], BF, tag="lr", bufs=2,
                                     name=f"lr{layer}_{cb}")
                        tb = txr[:]
                        lb = lr[:]
                        for h in range(HN):
                            kh = ks[h]
                            if kh > 0:
                                ap_i = bass.AP(tb.tensor, tb.offset + h * CW,
                                               [list(tb.ap[0]), [F, CPC], [1, kh]])
                                ap_o = bass.AP(lb.tensor, lb.offset + h * CW,
                                               [list(lb.ap[0]), [F, CPC], [1, kh]])
                                nc.scalar.activation(out=ap_o, in_=ap_i,
                                                     func=AF.Prelu, alpha=0.2)
                            if kh < CW:
                                off = h * CW + kh
                                ap_i = bass.AP(tb.tensor, tb.offset + off,
                                               [list(tb.ap[0]), [F, CPC],
                                                [1, CW - kh]])
                                ap_o = bass.AP(lb.tensor, lb.offset + off,
                                               [list(lb.ap[0]), [F, CPC],
                                                [1, CW - kh]])
                                nc.scalar.activation(out=ap_o, in_=ap_i,
                                                     func=AF.Prelu, alpha=5.0,
                                                     scale=0.2)
                        # alpha = per-head sum
                        al = wp.tile([P, CPC * HN], F32, tag="al", bufs=2,
                                     name=f"al{layer}_{cb}")
                        nc.vector.reduce_sum(
                            out=al[:],
                            in_=lr[:].rearrange("p c (h s) -> p c h s", s=CW),
                            axis=AX)
                        # p = exp(alpha) (bf16)
                        pe = wp.tile([P, CPC * HN], BF, tag="pe", bufs=2,
                                     name=f"pe{layer}_{cb}")
                        nc.scalar.activation(out=pe[:], in_=al[:], func=AF.Exp)
                        # thi = bf16(al + 8): snaps al to the exact bf16 grid
                        thi = wp.tile([P, CPC * HN], BF, tag="thi", bufs=2,
                                      name=f"thi{layer}_{cb}")
                        nc.scalar.activation(out=thi[:], in_=al[:], func=AF.Copy,
                                             bias=8.0)
                        # rhs = [p*xg | p | hi | lo]
                        rhs = wp.tile([P, CPC, RWB], BF, tag="rhs", bufs=3,
                                      name=f"rhs{layer}_{cb}")
                        rb = rhs[:]
                        xb = xg[:, sub * CPC:(sub + 1) * CPC, :]
                        pb = pe[:]
                        nc.vector.tensor_tensor(
                            out=_ap(rb, [rb.ap[0], [RWB, CPC], [CW, HN], [1, CW]]),
                            in0=_ap(xb, [xb.ap[0], [F, CPC], [CW, HN], [1, CW]]),
                            in1=_ap(pb, [pb.ap[0], [HN, CPC], [1, HN], [0, CW]]),
                            op=mul)
                        pc_out = bass.AP(rb.tensor, rb.offset + F,
                                         [list(rb.ap[0]), [RWB, CPC], [1, HN]])
                        nc.scalar.activation(
                            out=pc_out, in_=pb.rearrange("p (c h) -> p c h", h=HN),
                            func=AF.Copy)
                        hi_out = bass.AP(rb.tensor, rb.offset + F + HN,
                                         [list(rb.ap[0]), [RWB, CPC], [1, HN]])
                        nc.scalar.activation(
                            out=hi_out, in_=thi[:].rearrange(
                                "p (c h) -> p c h", h=HN),
                            func=AF.Copy, bias=-8.0)
                        lo_out = bass.AP(rb.tensor, rb.offset + F + 2 * HN,
                                         [list(rb.ap[0]), [RWB, CPC], [1, HN]])
                        nc.vector.tensor_tensor(
                            out=lo_out,
                            in0=al[:].rearrange("p (c h) -> p c h", h=HN),
                            in1=hi_out, op=mybir.AluOpType.subtract)
                        # scatter matmuls, PSUM-accumulated per block segment
                        for c in range(CPC):
                            ci = cb + c
                            blk = int(chunk_block[ci])
                            seg_start = ci == 0 or chunk_block[ci - 1] != blk
                            seg_end = ci == CH - 1 or chunk_block[ci + 1] != blk
                            if seg_start:
                                bp = pp.tile([P, RWB], F32, tag="bp", bufs=2,
                                             name=f"bp{layer}_{ci}")
                            nc.tensor.matmul(
                                out=bp[:], lhsT=qt[:, c, :], rhs=rhs[:, c, :],
                                start=seg_start, stop=seg_end)
                            if seg_end:
                                nc.vector.tensor_add(
                                    out=hacc[:, blk * RW:blk * RW + RWB],
                                    in0=hacc[:, blk * RW:blk * RW + RWB],
                                    in1=bp[:])

            def epilogue(layer, recip_sb, bb_sb):
                HN = cfg.H1 if layer == 1 else 1
                CW = F // HN
                NB = NBLK
                hb = hacc[:]
                # batched across all blocks: sa = hi + lo, eps, den, rec
                sa = wp.tile([P, NB, HN], F32, tag="sa", bufs=1,
                             name=f"sa{layer}")
                nc.vector.tensor_add(
                    out=sa[:],
                    in0=_ap(hb, [hb.ap[0], [RW, NB], [1, HN]], F + HN),
                    in1=_ap(hb, [hb.ap[0], [RW, NB], [1, HN]], F + 2 * HN))
                eps = wp.tile([P, NB, HN], F32, tag="eps", bufs=1,
                              name=f"eps{layer}")
                nc.scalar.activation(out=eps[:], in_=sa[:], func=AF.Exp,
                                     bias=lneps_sb[:, 0:1])
                den = wp.tile([P, NB, HN], F32, tag="den", bufs=1,
                              name=f"den{layer}")
                nc.vector.tensor_add(
                    out=den[:], in0=_ap(hb, [hb.ap[0], [RW, NB], [1, HN]], F),
                    in1=eps[:])
                rec = wp.tile([P, NB, HN], F32, tag="rec", bufs=1,
                              name=f"rec{layer}")
                nc.vector.reciprocal(out=rec[:], in_=den[:])
                # sc[n, b, c] = rec[n, b, head(c)] * recip_att[c]
                sc = wp.tile([P, NB * F], F32, tag="sc", bufs=1,
                             name=f"sc{layer}")
                scb = sc[:]
                rcb = rec[:]
                rpb = recip_sb[:]
                nc.vector.tensor_tensor(
                    out=_ap(scb, [scb.ap[0], [F, NB], [CW, HN], [1, CW]]),
                    in0=_ap(rcb, [rcb.ap[0], [HN, NB], [1, HN], [0, CW]]),
                    in1=_ap(rpb, [rpb.ap[0], [0, NB], [CW, HN], [1, CW]]),
                    op=mul)
                # y = msg * sc + bias
                so2 = stage_o[:].rearrange("p (b f) -> p b f", f=F)
                nc.vector.tensor_tensor(
                    out=so2, in0=_ap(hb, [hb.ap[0], [RW, NB], [1, F]]),
                    in1=sc[:].rearrange("p (b f) -> p b f", f=F), op=mul)
                bbb = bb_sb[:]
                nc.vector.tensor_tensor(
                    out=so2, in0=so2,
                    in1=_ap(bbb, [bbb.ap[0], [0, NB], [1, F]]),
                    op=mybir.AluOpType.add)
                sob = stage_o[:]
                # elu(y) = relu(y) + exp(-relu(-y)) - 1
                # tmp reuse: stage (xl staging, dead) and xTs (dead after L1)
                nc.scalar.activation(out=stage[:], in_=sob, func=AF.Relu,
                                     scale=-1.0)
                nc.scalar.activation(out=xTs_sb[:], in_=stage[:], func=AF.Exp,
                                     scale=-1.0)
                nc.vector.tensor_scalar_max(out=sob, in0=sob, scalar1=0.0)
                nc.vector.tensor_add(out=sob, in0=sob, in1=xTs_sb[:])
                nc.vector.tensor_scalar_add(out=sob, in0=sob, scalar1=-1.0)
                if layer == 1:
                    for b in range(NBLK):
                        trh = pp.tile([P, 512], F32, tag="trp", bufs=2,
                                      name=f"trh{b}")
                        nc.tensor.transpose(out=trh[:, 0:P],
                                            in_=stage_o[:, b * F:(b + 1) * F],
                                            identity=identF_sb[:])
                        nc.scalar.activation(out=hT_sb[:, b * P:(b + 1) * P],
                                             in_=trh[:, 0:P], func=AF.Copy)

            # ---- layer 1 ----
            node_phase(xTs_sb, w1_sb, xr1_sb)
            dma_stage_to(xl1sh)
            nc.gpsimd.collective_compute(
                "AllGather", mybir.AluOpType.bypass,
                replica_groups=[list(range(8))],
                ins=[xl1sh[:]], outs=[xl1full[:]])
            edge_pass(1, xl1full, xr1_sb, k1)
            epilogue(1, recip1_sb, bb1_sb)
            # ---- layer 2 ----
            node_phase(hT_sb, w2_sb, xr2_sb)
            dma_stage_to(xl2sh)
            nc.gpsimd.collective_compute(
                "AllGather", mybir.AluOpType.bypass,
                replica_groups=[list(range(8))],
                ins=[xl2sh[:]], outs=[xl2full[:]])
            edge_pass(2, xl2full, xr2_sb, k2)
            epilogue(2, recip2_sb, bb2_sb)
            oo = out[:].rearrange("(b p) f -> p b f", p=P)
            ii = stage_o[:].rearrange("p (b f) -> p b f", f=F)
            nc.sync.dma_start(out=oo, in_=ii)

    nc.compile()
    return nc


# ---------------------------------------------------------------------------
# public entry point
# ---------------------------------------------------------------------------
_CACHE = {}
LAST_RESULTS = None


def _trace_enabled():
    import os
    return os.environ.get("GAT_TRACE", "") == "1"


def _install_trace_shim():
    """antenv.axon_hooks is absent in this image; recreate it so trace=True
    can capture NTFF profiles through the axon PJRT plugin."""
    import sys, types
    if "antenv.axon_hooks" in sys.modules:
        return
    try:
        mod = types.ModuleType("antenv.axon_hooks")
        mod._hook = None
        mod.set_axon_ntff_profile_hook = lambda h: setattr(mod, "_hook", h)
        mod.get_axon_ntff_profile_hook = lambda: mod._hook
        sys.modules["antenv.axon_hooks"] = mod
        import antenv
        antenv.axon_hooks = mod
        from trn_agent_boot.trn_boot import _ntff_profile_via_ctypes
        mod._hook = _ntff_profile_via_ctypes("/opt/axon/libaxon_pjrt.so")
        import concourse.bass_utils as bu
        bu.upload_artifacts = lambda tmpdir: str(tmpdir)
    except Exception:
        pass


def kernel(x, edge_index, Wl1, Wr1, att1, b1, Wl2, Wr2, att2, b2):
    global LAST_RESULTS
    from concourse.bass_utils import run_bass_kernel_spmd

    trace = _trace_enabled()
    if trace:
        _install_trace_shim()

    x = np.asarray(x, np.float32)
    edge_index = np.asarray(edge_index)
    N, E = x.shape[0], edge_index.shape[1]
    cfg = Cfg(N, E, nblk=49)

    per_core, struct = host_prep(cfg, x, edge_index)
    consts, meta = host_consts(cfg, Wl1, Wr1, att1, b1, Wl2, Wr2, att2, b2)

    key = (N, E, x.shape[1], struct["S_A"], struct["S_B"],
           meta["k1"], meta["k2"])
    if key not in _CACHE:
        _CACHE[key] = build_program(cfg, struct, meta["k1"], meta["k2"])
    nc = _CACHE[key]

    in_maps = []
    for k in range(8):
        m = dict(per_core[k])
        m.update(consts)
        in_maps.append(m)
    res = run_bass_kernel_spmd(nc, in_maps, core_ids=list(range(8)), trace=trace)
    LAST_RESULTS = res
    outs = [np.asarray(res.results[k]["out"]) for k in range(8)]
    full = np.concatenate(outs, axis=0)[:N].astype(np.float32)
    unperm = np.empty_like(full)
    unperm[:, meta["perm2"]] = full
    return unperm


# revision 44
# speedup vs baseline: 1.0571x; 1.0571x over previous
"""Trainium2 Bass kernel for a 2-layer GATv2 aggregator (N=50000, E=800000).

Self-contained: kernel(**inputs) takes full inputs, shards across 8
NeuronCores internally, returns the full (50000, 128) float32 output.

v2 strategy (8-core SPMD, dst-sharded):
- Channels permuted per head (positive-att first) and tables pre-scaled by the
  SIGNED att value: t~_c = att_c*(xl_c + xr_c).  Then
  att_c*leaky(t_c) = Prelu(t~_c; 0.2) for att_c>0 and min(t~, 0.2 t~)
  = Prelu(0.2*t~; 5) for att_c<0, so alpha = plain per-head sum of the
  ACT output -- one strided reduce, no per-edge att multiply.
- Messages aggregate Sum p*x~l (scaled); epilogue divides by att_c per
  channel (recip const tile).  Layer-2 weight rows pre-permuted; final
  output unpermuted on host.
- All edge-pass tiles bf16 (tables, one-hots, rhs);  eps term
  1e-16*exp(sum alpha) (replicates the oracle's segment_max-is-sum bug)
  accumulated via exact hi/lo bf16 split columns in the scatter matmul.
- dma_gather with prepare_only+trigger_dma so SWDGE desc-gen overlaps
  the DMA drain;  gather calls of 2048 edges (bf16 rows, 256 B).
"""
import numpy as np
import ml_dtypes

import concourse.bass as bass
import concourse.bacc as bacc
import concourse.mybir as mybir
from concourse.tile import TileContext

BF16 = ml_dtypes.bfloat16
F32 = mybir.dt.float32
BF = mybir.dt.bfloat16
F8 = mybir.dt.float8e4
I16 = mybir.dt.int16
PAD_DST = 200.0
P = 128
CPC = 8           # chunks per compute group
GPC = 8           # chunks per gather call (1024 idx = SWDGE ring max)
NI = GPC * 128    # indices per gather call
import os
USE_PREP = os.environ.get("GAT_PREP", "0") == "1"


class Cfg:
    def __init__(self, N, E, nblk, feat=128, heads1=2):
        self.N, self.E = N, E
        self.NBLK = nblk
        self.SHARD = nblk * P
        self.NPAD = 8 * self.SHARD
        assert self.NPAD >= N and self.NPAD % 256 == 0
        self.HALF = self.NPAD // 2
        assert self.HALF <= 32767
        self.F = feat
        self.H1 = heads1
        self.C1 = feat // heads1


def host_prep(cfg, x, edge_index):
    """Returns (per_core_inputs: list of dict, struct: dict)."""
    N, E = cfg.N, cfg.E
    src = np.concatenate([np.asarray(edge_index[0]), np.arange(N)]).astype(np.int64)
    dst = np.concatenate([np.asarray(edge_index[1]), np.arange(N)]).astype(np.int64)
    ET = src.shape[0]

    core = dst // cfg.SHARD
    block = (dst % cfg.SHARD) // P
    dloc = dst % P
    half = (src >= cfg.HALF).astype(np.int64)
    gval = (src - half * cfg.HALF).astype(np.int64)

    # group = (core, half, block); rank within group
    key = (core * 2 + half) * cfg.NBLK + block
    order = np.argsort(key, kind="stable")
    key_s = key[order]
    ngroups = 8 * 2 * cfg.NBLK
    cnt = np.bincount(key_s, minlength=ngroups)
    starts = np.zeros(ngroups + 1, np.int64)
    np.cumsum(cnt, out=starts[1:])
    rank = np.arange(ET) - starts[key_s]

    cnt3 = cnt.reshape(8, 2, cfg.NBLK)
    S_A = int(np.ceil(cnt3[:, 0, :].max() / P))
    S_B = int(np.ceil(cnt3[:, 1, :].max() / P))
    S_A, S_B = max(S_A, 1), max(S_B, 1)
    CHA = -(-(cfg.NBLK * S_A) // GPC) * GPC
    CHB = -(-(cfg.NBLK * S_B) // GPC) * GPC
    CH = CHA + CHB
    CALLS = CH // GPC

    chunk_half = np.zeros(CH, np.int64)
    chunk_block = np.zeros(CH, np.int64)
    for c in range(CH):
        if c < CHA:
            chunk_half[c] = 0
            chunk_block[c] = min(c // S_A, cfg.NBLK - 1)
        else:
            chunk_half[c] = 1
            chunk_block[c] = min((c - CHA) // S_B, cfg.NBLK - 1)

    gidx = np.zeros((8, CH, P), np.int16)
    dstl = np.full((8, CH, P), PAD_DST, np.float32)
    g_half = half[order]
    g_core = core[order]
    g_block = block[order]
    slot_base = np.where(g_half == 0, g_block * S_A, CHA + g_block * S_B)
    slot = slot_base + rank // P
    pos = rank % P
    gidx[g_core, slot, pos] = gval[order].astype(np.int16)
    dstl[g_core, slot, pos] = dloc[order].astype(np.float32)

    # host-built scatter one-hots qt[e, n] = (dstl == n), streamed to SBUF by
    # HWDGE DMA (keeps the build off DVE, which serializes with SWDGE gathers);
    # fp8 to halve the stream's SDMA traffic (0.0/1.0 are exact)
    F8NP = ml_dtypes.float8_e4m3
    qts = np.zeros((8, P, CH * P), F8NP)
    narr = np.arange(P, dtype=np.float32)
    for k in range(8):
        oh = (dstl[k][:, :, None] == narr[None, None, :])  # [CH, e, n]
        qts[k] = np.ascontiguousarray(
            oh.transpose(1, 0, 2).reshape(P, CH * P)).astype(F8NP)

    # wrap gather indices per call of NI: [NI//16,16].T -> [16, NI//16]
    gw = gidx.reshape(8, CALLS, NI // 16, 16).transpose(0, 1, 3, 2)
    gw = gw.transpose(0, 2, 1, 3).reshape(8, 16, CALLS * (NI // 16))
    gw = np.tile(gw, (1, 8, 1))  # replicate to 128 partitions

    struct = dict(S_A=S_A, S_B=S_B, CHA=CHA, CHB=CHB, CH=CH, CALLS=CALLS,
                  chunk_half=chunk_half, chunk_block=chunk_block)

    x_pad = np.zeros((cfg.NPAD, cfg.F), np.float32)
    x_pad[:N] = np.asarray(x, np.float32)

    per_core = []
    for k in range(8):
        per_core.append(dict(
            xTs=np.ascontiguousarray(
                x_pad[k * cfg.SHARD:(k + 1) * cfg.SHARD].T.astype(BF16)),
            gidx=np.ascontiguousarray(gw[k]),
            qts=qts[k],
        ))
    return per_core, struct


def _perm_layer(Wl, Wr, att):
    """Channel perm (positive att first per head) + signed-scale weights."""
    att = np.asarray(att, np.float32)
    H, C = att.shape
    perm = np.zeros((H, C), np.int64)
    k = np.zeros(H, np.int64)
    for h in range(H):
        pos = np.where(att[h] > 0)[0]
        neg = np.where(att[h] <= 0)[0]
        perm[h] = np.concatenate([pos, neg])
        k[h] = len(pos)
    att_p = np.take_along_axis(att, perm, axis=1)
    s = att_p.reshape(-1)                    # signed scale per (permuted) chan
    flat_perm = (perm + np.arange(H)[:, None] * C).reshape(-1)
    Wl_t = np.asarray(Wl, np.float32)[:, flat_perm] * s[None, :]
    Wr_t = np.asarray(Wr, np.float32)[:, flat_perm] * s[None, :]
    return Wl_t, Wr_t, s, k, flat_perm


def host_consts(cfg, Wl1, Wr1, att1, b1, Wl2, Wr2, att2, b2):
    f = cfg.F
    Wl1t, Wr1t, s1, k1, perm1 = _perm_layer(Wl1, Wr1, att1)
    # layer2 rows permuted by perm1 (its input h is in permuted-1 order)
    Wl2t, Wr2t, s2, k2, perm2 = _perm_layer(
        np.asarray(Wl2, np.float32)[perm1], np.asarray(Wr2, np.float32)[perm1],
        att2)
    c = {}
    c["w1"] = np.hstack([Wl1t, Wr1t]).astype(BF16)
    c["w2"] = np.hstack([Wl2t, Wr2t]).astype(BF16)
    c["recip1"] = np.tile((1.0 / s1).reshape(1, f), (P, 1)).astype(np.float32)
    c["recip2"] = np.tile((1.0 / s2).reshape(1, f), (P, 1)).astype(np.float32)
    c["bb1"] = np.tile(np.asarray(b1, np.float32)[perm1].reshape(1, f), (P, 1))
    c["bb2"] = np.tile(np.asarray(b2, np.float32)[perm2].reshape(1, f), (P, 1))
    c["identB"] = np.eye(P, dtype=np.float32).astype(BF16)
    c["identB8"] = np.eye(P, dtype=np.float32).astype(ml_dtypes.float8_e4m3)
    c["identF"] = np.eye(P, dtype=np.float32)
    meta = dict(k1=tuple(int(v) for v in k1), k2=tuple(int(v) for v in k2),
                perm2=perm2)
    return c, meta


def _ap(base, layout, extra_offset=0):
    return bass.AP(base.tensor, base.offset + extra_offset,
                   [list(d) for d in layout])


def build_program(cfg, struct, k1, k2):
    NBLK, SHARD, NPAD, HALF, F = cfg.NBLK, cfg.SHARD, cfg.NPAD, cfg.HALF, cfg.F
    CH, CALLS = struct["CH"], struct["CALLS"]
    chunk_half, chunk_block = struct["chunk_half"], struct["chunk_block"]
    RW = F + 8

    nc = bacc.Bacc("TRN2", target_bir_lowering=False, debug=False,
                   num_devices=8, num_swdge_queues=4)

    xTs = nc.dram_tensor("xTs", [P, SHARD], BF, kind="ExternalInput")
    gidx = nc.dram_tensor("gidx", [P, CALLS * (NI // 16)], I16, kind="ExternalInput")
    qts = nc.dram_tensor("qts", [P, CH * P], F8, kind="ExternalInput")
    w1 = nc.dram_tensor("w1", [P, 2 * F], BF, kind="ExternalInput")
    w2 = nc.dram_tensor("w2", [P, 2 * F], BF, kind="ExternalInput")
    recip1 = nc.dram_tensor("recip1", [P, F], F32, kind="ExternalInput")
    recip2 = nc.dram_tensor("recip2", [P, F], F32, kind="ExternalInput")
    bb1 = nc.dram_tensor("bb1", [P, F], F32, kind="ExternalInput")
    bb2 = nc.dram_tensor("bb2", [P, F], F32, kind="ExternalInput")
    identB = nc.dram_tensor("identB", [P, P], BF, kind="ExternalInput")
    identB8 = nc.dram_tensor("identB8", [P, P], F8, kind="ExternalInput")
    identF = nc.dram_tensor("identF", [P, P], F32, kind="ExternalInput")
    out = nc.dram_tensor("out", [SHARD, F], F32, kind="ExternalOutput")

    eq = mybir.AluOpType.is_equal
    mul = mybir.AluOpType.mult
    AF = mybir.ActivationFunctionType
    AX = mybir.AxisListType.X

    with TileContext(nc) as tc:
        with (
            tc.tile_pool(name="const", bufs=1) as cpool,
            tc.tile_pool(name="big", bufs=1) as bigp,
            tc.tile_pool(name="work", bufs=1) as wp,
            tc.tile_pool(name="psum", bufs=1, space="PSUM") as pp,
            tc.tile_pool(name="dram", bufs=1, space="DRAM") as dp,
        ):
            def load_const(t, shape, dt):
                s = cpool.tile(shape, dt, name=t.name + "_sb")
                nc.sync.dma_start(out=s[:], in_=t[:])
                return s
            w1_sb = load_const(w1, [P, 2 * F], BF)
            w2_sb = load_const(w2, [P, 2 * F], BF)
            recip1_sb = load_const(recip1, [P, F], F32)
            recip2_sb = load_const(recip2, [P, F], F32)
            bb1_sb = load_const(bb1, [P, F], F32)
            bb2_sb = load_const(bb2, [P, F], F32)
            identB_sb = load_const(identB, [P, P], BF)
            identB8_sb = load_const(identB8, [P, P], F8)
            identF_sb = load_const(identF, [P, P], F32)
            xTs_sb = bigp.tile([P, SHARD], BF, name="xTs_sb")
            nc.sync.dma_start(out=xTs_sb[:], in_=xTs[:])
            gidx_sb = bigp.tile([P, CALLS * (NI // 16)], I16, name="gidx_sb")
            nc.sync.dma_start(out=gidx_sb[:], in_=gidx[:])

            lneps_sb = cpool.tile([P, 1], F32, name="lneps_sb")
            nc.vector.memset(lneps_sb[:], float(np.log(1e-16)))
            xr1_sb = bigp.tile([P, SHARD], BF, name="xr1_sb")
            xr2_sb = bigp.tile([P, SHARD], BF, name="xr2_sb")
            hT_sb = bigp.tile([P, SHARD], BF, name="hT_sb")
            hacc = bigp.tile([P, NBLK * RW], F32, name="hacc")
            stage = bigp.tile([P, SHARD], BF, name="stage")      # xl (bf16)
            stage_o = bigp.tile([P, SHARD], F32, name="stage_o")  # epilogue f32

            xl1sh = dp.tile([SHARD, F], BF, name="xl1sh")
            xl1full = dp.tile([NPAD, F], BF, name="xl1full", addr_space="Shared")
            xl2sh = dp.tile([SHARD, F], BF, name="xl2sh")
            xl2full = dp.tile([NPAD, F], BF, name="xl2full", addr_space="Shared")

            dma_sem = nc.alloc_semaphore("gat_dma")

            def node_phase(src_sb, w_sb, xr_dst):
                for j in range(NBLK):
                    mm = pp.tile([P, 8, P], F32, tag="txr", bufs=2, name=f"mm{j}")
                    mf = mm[:].rearrange("p c f -> p (c f)")
                    nc.tensor.matmul(out=mf[:, 0:2 * F],
                                     lhsT=src_sb[:, j * P:(j + 1) * P],
                                     rhs=w_sb[:], start=True, stop=True)
                    nc.scalar.activation(out=stage[:, j * F:(j + 1) * F],
                                         in_=mf[:, 0:F], func=AF.Copy)
                    nc.scalar.activation(out=xr_dst[:, j * P:(j + 1) * P],
                                         in_=mf[:, F:2 * F], func=AF.Copy)

            def dma_stage_to(dram_tile):
                o = dram_tile[:].rearrange("(b p) f -> p b f", p=P)
                i = stage[:].rearrange("p (b f) -> p b f", f=F)
                nc.sync.dma_start(out=o, in_=i)

            def edge_pass(layer, table, xr_sb, ks):
                HN = cfg.H1 if layer == 1 else 1
                CW = F // HN
                RWB = F + 3 * HN
                nc.vector.memset(hacc[:], 0.0)
                bp = None
                for g in range(CALLS):
                    cb0 = g * GPC
                    hf = int(chunk_half[cb0])
                    tab = table[:][0:HALF, :] if hf == 0 else table[:][HALF:NPAD, :]
                    xg = wp.tile([P, GPC, F], BF, tag="xg", bufs=3,
                                 name=f"xg{layer}_{g}")
                    if USE_PREP:
                        nc.gpsimd.dma_gather(
                            out_ap=xg[:], in_ap=tab,
                            idxs_ap=gidx_sb[:, g * (NI // 16):(g + 1) * (NI // 16)],
                            num_idxs=NI, num_idxs_reg=NI, elem_size=F,
                            prepare_only=True, sem=dma_sem, queue_num=0)
                        nc.gpsimd.trigger_dma(count=None)
                    else:
                        nc.gpsimd.dma_gather(
                            out_ap=xg[:], in_ap=tab,
                            idxs_ap=gidx_sb[:, g * (NI // 16):(g + 1) * (NI // 16)],
                            num_idxs=NI, num_idxs_reg=NI, elem_size=F,
                            queue_num=g % 4)
                    for sub in range(GPC // CPC):
                        cb = cb0 + sub * CPC
                        # one-hot QT[e, n] (lhsT for scatter) streamed from host
                        qt = wp.tile([P, CPC, P], F8, tag="qt", bufs=3,
                                     name=f"qt{layer}_{cb}")
                        nc.sync.dma_start(
                            out=qt[:],
                            in_=qts[:, cb * P:(cb + CPC) * P].rearrange(
                                "p (c f) -> p c f", f=P))
                        # Q[n, e] one-hot = PE transpose of QT; copy to SBUF on ACT
                        # fp8 transpose writes with element step 2 (16-bit
                        # granularity); tile holds 2*CPC*P fp8 = 2048 B
                        trp = pp.tile([P, 2 * CPC * P], F8, tag="trp", bufs=2,
                                      name=f"trp{layer}_{cb}")
                        tpb = trp[:]
                        for c in range(CPC):
                            nc.tensor.transpose(
                                out=_ap(tpb, [tpb.ap[0], [2, P]], 2 * c * P),
                                in_=qt[:, c, :], identity=identB8_sb[:])
                        q = wp.tile([P, CPC, P], F8, tag="q", bufs=3,
                                    name=f"q{layer}_{cb}")
                        nc.scalar.activation(
                            out=q[:],
                            in_=_ap(tpb, [tpb.ap[0], [2 * P, CPC], [2, P]]),
                            func=AF.Copy)
                        # t~ = Q.T @ xr (+ xg via identity matmul), PSUM f32
                        txr = pp.tile([P, CPC, P], F32, tag="txr", bufs=2,
                                      name=f"txr{layer}_{cb}")
                        for c in range(CPC):
                            blk = int(chunk_block[cb + c])
                            nc.tensor.matmul(
                                out=txr[:, c, :], lhsT=q[:, c, :],
                                rhs=xr_sb[:, blk * P:(blk + 1) * P],
                                start=True, stop=False)
                            nc.tensor.matmul(
                                out=txr[:, c, :], lhsT=identB_sb[:],
                                rhs=xg[:, sub * CPC + c, :],
                                start=False, stop=True)
                        # att_c*leaky(t_c): Prelu(x;.2) pos block,
                        # Prelu(.2x;5)=min(x,.2x) neg block, per head
                        lr = wp.tile([P, CPC, F], BF, tag="lr", bufs=2,
                                     name=f"lr{layer}_{cb}")
                        tb = txr[:]
                        lb = lr[:]
                        for h in range(HN):
                            kh = ks[h]
                            if kh > 0:
                                ap_i = bass.AP(tb.tensor, tb.offset + h * CW,
                                               [list(tb.ap[0]), [F, CPC], [1, kh]])
                                ap_o = bass.AP(lb.tensor, lb.offset + h * CW,
                                               [list(lb.ap[0]), [F, CPC], [1, kh]])
                                nc.scalar.activation(out=ap_o, in_=ap_i,
                                                     func=AF.Prelu, alpha=0.2)
                            if kh < CW:
                                off = h * CW + kh
                                ap_i = bass.AP(tb.tensor, tb.offset + off,
                                               [list(tb.ap[0]), [F, CPC],
                                                [1, CW - kh]])
                                ap_o = bass.AP(lb.tensor, lb.offset + off,
                                               [list(lb.ap[0]), [F, CPC],
                                                [1, CW - kh]])
                                nc.scalar.activation(out=ap_o, in_=ap_i,
                                                     func=AF.Prelu, alpha=5.0,
                                                     scale=0.2)
                        # alpha = per-head sum
                        al = wp.tile([P, CPC * HN], F32, tag="al", bufs=2,
                                     name=f"al{layer}_{cb}")
                        nc.vector.reduce_sum(
                            out=al[:],
                            in_=lr[:].rearrange("p c (h s) -> p c h s", s=CW),
                            axis=AX)
                        # p = exp(alpha) (bf16)
                        pe = wp.tile([P, CPC * HN], BF, tag="pe", bufs=2,
                                     name=f"pe{layer}_{cb}")
                        nc.scalar.activation(out=pe[:], in_=al[:], func=AF.Exp)
                        # thi = bf16(al + 8): snaps al to the exact bf16 grid
                        thi = wp.tile([P, CPC * HN], BF, tag="thi", bufs=2,
                                      name=f"thi{layer}_{cb}")
                        nc.scalar.activation(out=thi[:], in_=al[:], func=AF.Copy,
                                             bias=8.0)
                        # rhs = [p*xg | p | hi | lo]
                        rhs = wp.tile([P, CPC, RWB], BF, tag="rhs", bufs=3,
                                      name=f"rhs{layer}_{cb}")
                        rb = rhs[:]
                        xb = xg[:, sub * CPC:(sub + 1) * CPC, :]
                        pb = pe[:]
                        nc.vector.tensor_tensor(
                            out=_ap(rb, [rb.ap[0], [RWB, CPC], [CW, HN], [1, CW]]),
                            in0=_ap(xb, [xb.ap[0], [F, CPC], [CW, HN], [1, CW]]),
                            in1=_ap(pb, [pb.ap[0], [HN, CPC], [1, HN], [0, CW]]),
                            op=mul)
                        pc_out = bass.AP(rb.tensor, rb.offset + F,
                                         [list(rb.ap[0]), [RWB, CPC], [1, HN]])
                        nc.scalar.activation(
                            out=pc_out, in_=pb.rearrange("p (c h) -> p c h", h=HN),
                            func=AF.Copy)
                        hi_out = bass.AP(rb.tensor, rb.offset + F + HN,
                                         [list(rb.ap[0]), [RWB, CPC], [1, HN]])
                        nc.scalar.activation(
                            out=hi_out, in_=thi[:].rearrange(
                                "p (c h) -> p c h", h=HN),
                            func=AF.Copy, bias=-8.0)
                        lo_out = bass.AP(rb.tensor, rb.offset + F + 2 * HN,
                                         [list(rb.ap[0]), [RWB, CPC], [1, HN]])
                        nc.vector.tensor_tensor(
                            out=lo_out,
                            in0=al[:].rearrange("p (c h) -> p c h", h=HN),
                            in1=hi_out, op=mybir.AluOpType.subtract)
                        # scatter matmuls, PSUM-accumulated per block segment
                        for c in range(CPC):
                            ci = cb + c
                            blk = int(chunk_block[ci])
                            seg_start = ci == 0 or chunk_block[ci - 1] != blk
                            seg_end = ci == CH - 1 or chunk_block[ci + 1] != blk
                            if seg_start:
                                bp = pp.tile([P, RWB], F32, tag="bp", bufs=2,
                                             name=f"bp{layer}_{ci}")
                            nc.tensor.matmul(
                                out=bp[:], lhsT=qt[:, c, :], rhs=rhs[:, c, :],
                                start=seg_start, stop=seg_end)
                            if seg_end:
                                nc.vector.tensor_add(
                                    out=hacc[:, blk * RW:blk * RW + RWB],
                                    in0=hacc[:, blk * RW:blk * RW + RWB],
                                    in1=bp[:])

            def epilogue(layer, recip_sb, bb_sb):
                HN = cfg.H1 if layer == 1 else 1
                CW = F // HN
                NB = NBLK
                hb = hacc[:]
                # batched across all blocks: sa = hi + lo, eps, den, rec
                sa = wp.tile([P, NB, HN], F32, tag="sa", bufs=1,
                             name=f"sa{layer}")
                nc.vector.tensor_add(
                    out=sa[:],
                    in0=_ap(hb, [hb.ap[0], [RW, NB], [1, HN]], F + HN),
                    in1=_ap(hb, [hb.ap[0], [RW, NB], [1, HN]], F + 2 * HN))
                eps = wp.tile([P, NB, HN], F32, tag="eps", bufs=1,
                              name=f"eps{layer}")
                nc.scalar.activation(out=eps[:], in_=sa[:], func=AF.Exp,
                                     bias=lneps_sb[:, 0:1])
                den = wp.tile([P, NB, HN], F32, tag="den", bufs=1,
                              name=f"den{layer}")
                nc.vector.tensor_add(
                    out=den[:], in0=_ap(hb, [hb.ap[0], [RW, NB], [1, HN]], F),
                    in1=eps[:])
                rec = wp.tile([P, NB, HN], F32, tag="rec", bufs=1,
                              name=f"rec{layer}")
                nc.vector.reciprocal(out=rec[:], in_=den[:])
                # sc[n, b, c] = rec[n, b, head(c)] * recip_att[c]
                sc = wp.tile([P, NB * F], F32, tag="sc", bufs=1,
                             name=f"sc{layer}")
                scb = sc[:]
                rcb = rec[:]
                rpb = recip_sb[:]
                nc.vector.tensor_tensor(
                    out=_ap(scb, [scb.ap[0], [F, NB], [CW, HN], [1, CW]]),
                    in0=_ap(rcb, [rcb.ap[0], [HN, NB], [1, HN], [0, CW]]),
                    in1=_ap(rpb, [rpb.ap[0], [0, NB], [CW, HN], [1, CW]]),
                    op=mul)
                # y = msg * sc + bias
                so2 = stage_o[:].rearrange("p (b f) -> p b f", f=F)
                nc.vector.tensor_tensor(
                    out=so2, in0=_ap(hb, [hb.ap[0], [RW, NB], [1, F]]),
                    in1=sc[:].rearrange("p (b f) -> p b f", f=F), op=mul)
                bbb = bb_sb[:]
                nc.vector.tensor_tensor(
                    out=so2, in0=so2,
                    in1=_ap(bbb, [bbb.ap[0], [0, NB], [1, F]]),
                    op=mybir.AluOpType.add)
                sob = stage_o[:]
                # elu(y) = relu(y) + exp(-relu(-y)) - 1
                # tmp reuse: stage (xl staging, dead) and xTs (dead after L1)
                nc.scalar.activation(out=stage[:], in_=sob, func=AF.Relu,
                                     scale=-1.0)
                nc.scalar.activation(out=xTs_sb[:], in_=stage[:], func=AF.Exp,
                                     scale=-1.0)
                nc.vector.tensor_scalar_max(out=sob, in0=sob, scalar1=0.0)
                nc.vector.tensor_add(out=sob, in0=sob, in1=xTs_sb[:])
                nc.vector.tensor_scalar_add(out=sob, in0=sob, scalar1=-1.0)
                if layer == 1:
                    for b in range(NBLK):
                        trh = pp.tile([P, 512], F32, tag="trp", bufs=2,
                                      name=f"trh{b}")
                        nc.tensor.transpose(out=trh[:, 0:P],
                                            in_=stage_o[:, b * F:(b + 1) * F],
                                            identity=identF_sb[:])
                        nc.scalar.activation(out=hT_sb[:, b * P:(b + 1) * P],
                                             in_=trh[:, 0:P], func=AF.Copy)

            # ---- layer 1 ----
            node_phase(xTs_sb, w1_sb, xr1_sb)
            dma_stage_to(xl1sh)
            nc.gpsimd.collective_compute(
                "AllGather", mybir.AluOpType.bypass,
                replica_groups=[list(range(8))],
                ins=[xl1sh[:]], outs=[xl1full[:]])
            edge_pass(1, xl1full, xr1_sb, k1)
            epilogue(1, recip1_sb, bb1_sb)
            # ---- layer 2 ----
            node_phase(hT_sb, w2_sb, xr2_sb)
            dma_stage_to(xl2sh)
            nc.gpsimd.collective_compute(
                "AllGather", mybir.AluOpType.bypass,
                replica_groups=[list(range(8))],
                ins=[xl2sh[:]], outs=[xl2full[:]])
            edge_pass(2, xl2full, xr2_sb, k2)
            epilogue(2, recip2_sb, bb2_sb)
            oo = out[:].rearrange("(b p) f -> p b f", p=P)
            ii = stage_o[:].rearrange("p (b f) -> p b f", f=F)
            nc.sync.dma_start(out=oo, in_=ii)

    nc.compile()
    return nc


# ---------------------------------------------------------------------------
# public entry point
# ---------------------------------------------------------------------------
_CACHE = {}
LAST_RESULTS = None


def _trace_enabled():
    import os
    return os.environ.get("GAT_TRACE", "") == "1"


def _install_trace_shim():
    """antenv.axon_hooks is absent in this image; recreate it so trace=True
    can capture NTFF profiles through the axon PJRT plugin."""
    import sys, types
    if "antenv.axon_hooks" in sys.modules:
        return
    try:
        mod = types.ModuleType("antenv.axon_hooks")
        mod._hook = None
        mod.set_axon_ntff_profile_hook = lambda h: setattr(mod, "_hook", h)
        mod.get_axon_ntff_profile_hook = lambda: mod._hook
        sys.modules["antenv.axon_hooks"] = mod
        import antenv
        antenv.axon_hooks = mod
        from trn_agent_boot.trn_boot import _ntff_profile_via_ctypes
        mod._hook = _ntff_profile_via_ctypes("/opt/axon/libaxon_pjrt.so")
        import concourse.bass_utils as bu
        bu.upload_artifacts = lambda tmpdir: str(tmpdir)
    except Exception:
        pass


def kernel(x, edge_index, Wl1, Wr1, att1, b1, Wl2, Wr2, att2, b2):
    global LAST_RESULTS
    from concourse.bass_utils import run_bass_kernel_spmd

    trace = _trace_enabled()
    if trace:
        _install_trace_shim()

    x = np.asarray(x, np.float32)
    edge_index = np.asarray(edge_index)
    N, E = x.shape[0], edge_index.shape[1]
    cfg = Cfg(N, E, nblk=49)

    per_core, struct = host_prep(cfg, x, edge_index)
    consts, meta = host_consts(cfg, Wl1, Wr1, att1, b1, Wl2, Wr2, att2, b2)

    key = (N, E, x.shape[1], struct["S_A"], struct["S_B"],
           meta["k1"], meta["k2"])
    if key not in _CACHE:
        _CACHE[key] = build_program(cfg, struct, meta["k1"], meta["k2"])
    nc = _CACHE[key]

    in_maps = []
    for k in range(8):
        m = dict(per_core[k])
        m.update(consts)
        in_maps.append(m)
    res = run_bass_kernel_spmd(nc, in_maps, core_ids=list(range(8)), trace=trace)
    LAST_RESULTS = res
    outs = [np.asarray(res.results[k]["out"]) for k in range(8)]
    full = np.concatenate(outs, axis=0)[:N].astype(np.float32)
    unperm = np.empty_like(full)
    unperm[:, meta["perm2"]] = full
    return unperm


# revision 45
# speedup vs baseline: 1.2357x; 1.1689x over previous
"""Trainium2 Bass kernel for a 2-layer GATv2 aggregator (N=50000, E=800000).

Self-contained: kernel(**inputs) takes full inputs, shards across 8
NeuronCores internally, returns the full (50000, 128) float32 output.

v2 strategy (8-core SPMD, dst-sharded):
- Channels permuted per head (positive-att first) and tables pre-scaled by the
  SIGNED att value: t~_c = att_c*(xl_c + xr_c).  Then
  att_c*leaky(t_c) = Prelu(t~_c; 0.2) for att_c>0 and min(t~, 0.2 t~)
  = Prelu(0.2*t~; 5) for att_c<0, so alpha = plain per-head sum of the
  ACT output -- one strided reduce, no per-edge att multiply.
- Messages aggregate Sum p*x~l (scaled); epilogue divides by att_c per
  channel (recip const tile).  Layer-2 weight rows pre-permuted; final
  output unpermuted on host.
- All edge-pass tiles bf16 (tables, one-hots, rhs);  eps term
  1e-16*exp(sum alpha) (replicates the oracle's segment_max-is-sum bug)
  accumulated via exact hi/lo bf16 split columns in the scatter matmul.
- dma_gather with prepare_only+trigger_dma so SWDGE desc-gen overlaps
  the DMA drain;  gather calls of 2048 edges (bf16 rows, 256 B).
"""
import numpy as np
import ml_dtypes

import concourse.bass as bass
import concourse.bacc as bacc
import concourse.mybir as mybir
from concourse.tile import TileContext

BF16 = ml_dtypes.bfloat16
F32 = mybir.dt.float32
BF = mybir.dt.bfloat16
F8 = mybir.dt.float8e4
I16 = mybir.dt.int16
PAD_DST = 200.0
P = 128
CPC = 8           # chunks per compute group
GPC = 8           # chunks per gather call (1024 idx = SWDGE ring max)
NI = GPC * 128    # indices per gather call
import os
USE_PREP = os.environ.get("GAT_PREP", "0") == "1"


class Cfg:
    def __init__(self, N, E, nblk, feat=128, heads1=2):
        self.N, self.E = N, E
        self.NBLK = nblk
        self.SHARD = nblk * P
        self.NPAD = 8 * self.SHARD
        assert self.NPAD >= N and self.NPAD % 256 == 0
        self.HALF = self.NPAD // 2
        assert self.HALF <= 32767
        self.F = feat
        self.H1 = heads1
        self.C1 = feat // heads1


def host_prep(cfg, x, edge_index):
    """Returns (per_core_inputs: list of dict, struct: dict)."""
    N, E = cfg.N, cfg.E
    src = np.concatenate([np.asarray(edge_index[0]), np.arange(N)]).astype(np.int64)
    dst = np.concatenate([np.asarray(edge_index[1]), np.arange(N)]).astype(np.int64)
    ET = src.shape[0]

    core = dst // cfg.SHARD
    block = (dst % cfg.SHARD) // P
    dloc = dst % P
    half = (src >= cfg.HALF).astype(np.int64)
    gval = (src - half * cfg.HALF).astype(np.int64)

    # group = (core, half, block); rank within group
    key = (core * 2 + half) * cfg.NBLK + block
    order = np.argsort(key, kind="stable")
    key_s = key[order]
    ngroups = 8 * 2 * cfg.NBLK
    cnt = np.bincount(key_s, minlength=ngroups)
    starts = np.zeros(ngroups + 1, np.int64)
    np.cumsum(cnt, out=starts[1:])
    rank = np.arange(ET) - starts[key_s]

    cnt3 = cnt.reshape(8, 2, cfg.NBLK)
    S_A = int(np.ceil(cnt3[:, 0, :].max() / P))
    S_B = int(np.ceil(cnt3[:, 1, :].max() / P))
    S_A, S_B = max(S_A, 1), max(S_B, 1)
    CHA = -(-(cfg.NBLK * S_A) // GPC) * GPC
    CHB = -(-(cfg.NBLK * S_B) // GPC) * GPC
    CH = CHA + CHB
    CALLS = CH // GPC

    chunk_half = np.zeros(CH, np.int64)
    chunk_block = np.zeros(CH, np.int64)
    for c in range(CH):
        if c < CHA:
            chunk_half[c] = 0
            chunk_block[c] = min(c // S_A, cfg.NBLK - 1)
        else:
            chunk_half[c] = 1
            chunk_block[c] = min((c - CHA) // S_B, cfg.NBLK - 1)

    gidx = np.zeros((8, CH, P), np.int16)
    dstl = np.full((8, CH, P), PAD_DST, np.float32)
    g_half = half[order]
    g_core = core[order]
    g_block = block[order]
    slot_base = np.where(g_half == 0, g_block * S_A, CHA + g_block * S_B)
    slot = slot_base + rank // P
    pos = rank % P
    gidx[g_core, slot, pos] = gval[order].astype(np.int16)
    dstl[g_core, slot, pos] = dloc[order].astype(np.float32)

    # host-built scatter one-hots qt[e, n] = (dstl == n), streamed to SBUF by
    # HWDGE DMA (keeps the build off DVE, which serializes with SWDGE gathers);
    # fp8 to halve the stream's SDMA traffic (0.0/1.0 are exact)
    F8NP = ml_dtypes.float8_e4m3
    qts = np.zeros((8, P, CH * P), F8NP)
    narr = np.arange(P, dtype=np.float32)
    for k in range(8):
        oh = (dstl[k][:, :, None] == narr[None, None, :])  # [CH, e, n]
        qts[k] = np.ascontiguousarray(
            oh.transpose(1, 0, 2).reshape(P, CH * P)).astype(F8NP)

    # wrap gather indices per call of NI: [NI//16,16].T -> [16, NI//16]
    gw = gidx.reshape(8, CALLS, NI // 16, 16).transpose(0, 1, 3, 2)
    gw = gw.transpose(0, 2, 1, 3).reshape(8, 16, CALLS * (NI // 16))
    gw = np.tile(gw, (1, 8, 1))  # replicate to 128 partitions

    struct = dict(S_A=S_A, S_B=S_B, CHA=CHA, CHB=CHB, CH=CH, CALLS=CALLS,
                  chunk_half=chunk_half, chunk_block=chunk_block)

    x_pad = np.zeros((cfg.NPAD, cfg.F), np.float32)
    x_pad[:N] = np.asarray(x, np.float32)

    per_core = []
    for k in range(8):
        per_core.append(dict(
            xTs=np.ascontiguousarray(
                x_pad[k * cfg.SHARD:(k + 1) * cfg.SHARD].T.astype(BF16)),
            gidx=np.ascontiguousarray(gw[k]),
            qts=qts[k],
        ))
    return per_core, struct


def _perm_layer(Wl, Wr, att):
    """Channel perm (positive att first per head) + signed-scale weights."""
    att = np.asarray(att, np.float32)
    H, C = att.shape
    perm = np.zeros((H, C), np.int64)
    k = np.zeros(H, np.int64)
    for h in range(H):
        pos = np.where(att[h] > 0)[0]
        neg = np.where(att[h] <= 0)[0]
        perm[h] = np.concatenate([pos, neg])
        k[h] = len(pos)
    att_p = np.take_along_axis(att, perm, axis=1)
    s = att_p.reshape(-1)                    # signed scale per (permuted) chan
    flat_perm = (perm + np.arange(H)[:, None] * C).reshape(-1)
    Wl_t = np.asarray(Wl, np.float32)[:, flat_perm] * s[None, :]
    Wr_t = np.asarray(Wr, np.float32)[:, flat_perm] * s[None, :]
    return Wl_t, Wr_t, s, k, flat_perm


def host_consts(cfg, Wl1, Wr1, att1, b1, Wl2, Wr2, att2, b2):
    f = cfg.F
    Wl1t, Wr1t, s1, k1, perm1 = _perm_layer(Wl1, Wr1, att1)
    # layer2 rows permuted by perm1 (its input h is in permuted-1 order)
    Wl2t, Wr2t, s2, k2, perm2 = _perm_layer(
        np.asarray(Wl2, np.float32)[perm1], np.asarray(Wr2, np.float32)[perm1],
        att2)
    c = {}
    c["w1"] = np.hstack([Wl1t, Wr1t]).astype(BF16)
    c["w2"] = np.hstack([Wl2t, Wr2t]).astype(BF16)
    c["recip1"] = np.tile((1.0 / s1).reshape(1, f), (P, 1)).astype(np.float32)
    c["recip2"] = np.tile((1.0 / s2).reshape(1, f), (P, 1)).astype(np.float32)
    c["bb1"] = np.tile(np.asarray(b1, np.float32)[perm1].reshape(1, f), (P, 1))
    c["bb2"] = np.tile(np.asarray(b2, np.float32)[perm2].reshape(1, f), (P, 1))
    c["identB"] = np.eye(P, dtype=np.float32).astype(BF16)
    c["identB8"] = np.eye(P, dtype=np.float32).astype(ml_dtypes.float8_e4m3)
    c["identF"] = np.eye(P, dtype=np.float32)
    meta = dict(k1=tuple(int(v) for v in k1), k2=tuple(int(v) for v in k2),
                perm2=perm2)
    return c, meta


def _ap(base, layout, extra_offset=0):
    return bass.AP(base.tensor, base.offset + extra_offset,
                   [list(d) for d in layout])


def build_program(cfg, struct, k1, k2):
    NBLK, SHARD, NPAD, HALF, F = cfg.NBLK, cfg.SHARD, cfg.NPAD, cfg.HALF, cfg.F
    CH, CALLS = struct["CH"], struct["CALLS"]
    chunk_half, chunk_block = struct["chunk_half"], struct["chunk_block"]
    RW = F + 8

    nc = bacc.Bacc("TRN2", target_bir_lowering=False, debug=False,
                   num_devices=8, num_swdge_queues=4)

    xTs = nc.dram_tensor("xTs", [P, SHARD], BF, kind="ExternalInput")
    gidx = nc.dram_tensor("gidx", [P, CALLS * (NI // 16)], I16, kind="ExternalInput")
    qts = nc.dram_tensor("qts", [P, CH * P], F8, kind="ExternalInput")
    w1 = nc.dram_tensor("w1", [P, 2 * F], BF, kind="ExternalInput")
    w2 = nc.dram_tensor("w2", [P, 2 * F], BF, kind="ExternalInput")
    recip1 = nc.dram_tensor("recip1", [P, F], F32, kind="ExternalInput")
    recip2 = nc.dram_tensor("recip2", [P, F], F32, kind="ExternalInput")
    bb1 = nc.dram_tensor("bb1", [P, F], F32, kind="ExternalInput")
    bb2 = nc.dram_tensor("bb2", [P, F], F32, kind="ExternalInput")
    identB = nc.dram_tensor("identB", [P, P], BF, kind="ExternalInput")
    identB8 = nc.dram_tensor("identB8", [P, P], F8, kind="ExternalInput")
    identF = nc.dram_tensor("identF", [P, P], F32, kind="ExternalInput")
    out = nc.dram_tensor("out", [SHARD, F], F32, kind="ExternalOutput")

    eq = mybir.AluOpType.is_equal
    mul = mybir.AluOpType.mult
    AF = mybir.ActivationFunctionType
    AX = mybir.AxisListType.X

    with TileContext(nc) as tc:
        with (
            tc.tile_pool(name="const", bufs=1) as cpool,
            tc.tile_pool(name="big", bufs=1) as bigp,
            tc.tile_pool(name="work", bufs=1) as wp,
            tc.tile_pool(name="psum", bufs=1, space="PSUM") as pp,
            tc.tile_pool(name="dram", bufs=1, space="DRAM") as dp,
        ):
            def load_const(t, shape, dt):
                s = cpool.tile(shape, dt, name=t.name + "_sb")
                nc.sync.dma_start(out=s[:], in_=t[:])
                return s
            w1_sb = load_const(w1, [P, 2 * F], BF)
            w2_sb = load_const(w2, [P, 2 * F], BF)
            recip1_sb = load_const(recip1, [P, F], F32)
            recip2_sb = load_const(recip2, [P, F], F32)
            bb1_sb = load_const(bb1, [P, F], F32)
            bb2_sb = load_const(bb2, [P, F], F32)
            identB_sb = load_const(identB, [P, P], BF)
            identB8_sb = load_const(identB8, [P, P], F8)
            identF_sb = load_const(identF, [P, P], F32)
            xTs_sb = bigp.tile([P, SHARD], BF, name="xTs_sb")
            nc.sync.dma_start(out=xTs_sb[:], in_=xTs[:])
            gidx_sb = bigp.tile([P, CALLS * (NI // 16)], I16, name="gidx_sb")
            nc.sync.dma_start(out=gidx_sb[:], in_=gidx[:])

            lneps_sb = cpool.tile([P, 1], F32, name="lneps_sb")
            nc.vector.memset(lneps_sb[:], float(np.log(1e-16)))
            xr1_sb = bigp.tile([P, SHARD], BF, name="xr1_sb")
            xr2_sb = bigp.tile([P, SHARD], BF, name="xr2_sb")
            hT_sb = bigp.tile([P, SHARD], BF, name="hT_sb")
            hacc = bigp.tile([P, NBLK * RW], F32, name="hacc")
            stage = bigp.tile([P, SHARD], BF, name="stage")      # xl (bf16)
            stage_o = bigp.tile([P, SHARD], F32, name="stage_o")  # epilogue f32

            xl1sh = dp.tile([SHARD, F], BF, name="xl1sh")
            xl1full = dp.tile([NPAD, F], BF, name="xl1full", addr_space="Shared")
            xl2sh = dp.tile([SHARD, F], BF, name="xl2sh")
            xl2full = dp.tile([NPAD, F], BF, name="xl2full", addr_space="Shared")

            dma_sem = nc.alloc_semaphore("gat_dma")

            def node_phase(src_sb, w_sb, xr_dst):
                for j in range(NBLK):
                    mm = pp.tile([P, 8, P], F32, tag="txr", bufs=2, name=f"mm{j}")
                    mf = mm[:].rearrange("p c f -> p (c f)")
                    nc.tensor.matmul(out=mf[:, 0:2 * F],
                                     lhsT=src_sb[:, j * P:(j + 1) * P],
                                     rhs=w_sb[:], start=True, stop=True)
                    nc.scalar.activation(out=stage[:, j * F:(j + 1) * F],
                                         in_=mf[:, 0:F], func=AF.Copy)
                    nc.scalar.activation(out=xr_dst[:, j * P:(j + 1) * P],
                                         in_=mf[:, F:2 * F], func=AF.Copy)

            def dma_stage_to(dram_tile):
                o = dram_tile[:].rearrange("(b p) f -> p b f", p=P)
                i = stage[:].rearrange("p (b f) -> p b f", f=F)
                nc.sync.dma_start(out=o, in_=i)

            def edge_pass(layer, table, xr_sb, ks):
                HN = cfg.H1 if layer == 1 else 1
                CW = F // HN
                RWB = F + 3 * HN
                nc.vector.memset(hacc[:], 0.0)
                bp = None
                for g in range(CALLS):
                    cb0 = g * GPC
                    hf = int(chunk_half[cb0])
                    tab = table[:][0:HALF, :] if hf == 0 else table[:][HALF:NPAD, :]
                    xg = wp.tile([P, GPC, F], BF, tag="xg", bufs=4,
                                 name=f"xg{layer}_{g}")
                    if USE_PREP:
                        nc.gpsimd.dma_gather(
                            out_ap=xg[:], in_ap=tab,
                            idxs_ap=gidx_sb[:, g * (NI // 16):(g + 1) * (NI // 16)],
                            num_idxs=NI, num_idxs_reg=NI, elem_size=F,
                            prepare_only=True, sem=dma_sem, queue_num=0)
                        nc.gpsimd.trigger_dma(count=None)
                    else:
                        nc.gpsimd.dma_gather(
                            out_ap=xg[:], in_ap=tab,
                            idxs_ap=gidx_sb[:, g * (NI // 16):(g + 1) * (NI // 16)],
                            num_idxs=NI, num_idxs_reg=NI, elem_size=F,
                            queue_num=g % 4)
                    for sub in range(GPC // CPC):
                        cb = cb0 + sub * CPC
                        # one-hot QT[e, n] (lhsT for scatter) streamed from host
                        qt = wp.tile([P, CPC, P], F8, tag="qt", bufs=4,
                                     name=f"qt{layer}_{cb}")
                        nc.sync.dma_start(
                            out=qt[:],
                            in_=qts[:, cb * P:(cb + CPC) * P].rearrange(
                                "p (c f) -> p c f", f=P))
                        # Q[n, e] one-hot = PE transpose of QT; copy to SBUF on ACT
                        # fp8 transpose writes with element step 2 (16-bit
                        # granularity); tile holds 2*CPC*P fp8 = 2048 B
                        trp = pp.tile([P, 2 * CPC * P], F8, tag="trp", bufs=2,
                                      name=f"trp{layer}_{cb}")
                        tpb = trp[:]
                        for c in range(CPC):
                            nc.tensor.transpose(
                                out=_ap(tpb, [tpb.ap[0], [2, P]], 2 * c * P),
                                in_=qt[:, c, :], identity=identB8_sb[:])
                        q = wp.tile([P, CPC, P], F8, tag="q", bufs=4,
                                    name=f"q{layer}_{cb}")
                        nc.scalar.activation(
                            out=q[:],
                            in_=_ap(tpb, [tpb.ap[0], [2 * P, CPC], [2, P]]),
                            func=AF.Copy)
                        # t~ = Q.T @ xr (+ xg via identity matmul), PSUM f32
                        txr = pp.tile([P, CPC, P], F32, tag="txr", bufs=2,
                                      name=f"txr{layer}_{cb}")
                        for c in range(CPC):
                            blk = int(chunk_block[cb + c])
                            nc.tensor.matmul(
                                out=txr[:, c, :], lhsT=q[:, c, :],
                                rhs=xr_sb[:, blk * P:(blk + 1) * P],
                                start=True, stop=False)
                            nc.tensor.matmul(
                                out=txr[:, c, :], lhsT=identB_sb[:],
                                rhs=xg[:, sub * CPC + c, :],
                                start=False, stop=True)
                        # att_c*leaky(t_c): Prelu(x;.2) pos block,
                        # Prelu(.2x;5)=min(x,.2x) neg block, per head
                        lr = wp.tile([P, CPC, F], BF, tag="lr", bufs=3,
                                     name=f"lr{layer}_{cb}")
                        tb = txr[:]
                        lb = lr[:]
                        for h in range(HN):
                            kh = ks[h]
                            if kh > 0:
                                ap_i = bass.AP(tb.tensor, tb.offset + h * CW,
                                               [list(tb.ap[0]), [F, CPC], [1, kh]])
                                ap_o = bass.AP(lb.tensor, lb.offset + h * CW,
                                               [list(lb.ap[0]), [F, CPC], [1, kh]])
                                nc.scalar.activation(out=ap_o, in_=ap_i,
                                                     func=AF.Prelu, alpha=0.2)
                            if kh < CW:
                                off = h * CW + kh
                                ap_i = bass.AP(tb.tensor, tb.offset + off,
                                               [list(tb.ap[0]), [F, CPC],
                                                [1, CW - kh]])
                                ap_o = bass.AP(lb.tensor, lb.offset + off,
                                               [list(lb.ap[0]), [F, CPC],
                                                [1, CW - kh]])
                                nc.scalar.activation(out=ap_o, in_=ap_i,
                                                     func=AF.Prelu, alpha=5.0,
                                                     scale=0.2)
                        # alpha = per-head sum
                        al = wp.tile([P, CPC * HN], F32, tag="al", bufs=3,
                                     name=f"al{layer}_{cb}")
                        nc.vector.reduce_sum(
                            out=al[:],
                            in_=lr[:].rearrange("p c (h s) -> p c h s", s=CW),
                            axis=AX)
                        # p = exp(alpha) (bf16)
                        pe = wp.tile([P, CPC * HN], BF, tag="pe", bufs=3,
                                     name=f"pe{layer}_{cb}")
                        nc.scalar.activation(out=pe[:], in_=al[:], func=AF.Exp)
                        # thi = bf16(al + 8): snaps al to the exact bf16 grid
                        thi = wp.tile([P, CPC * HN], BF, tag="thi", bufs=3,
                                      name=f"thi{layer}_{cb}")
                        nc.scalar.activation(out=thi[:], in_=al[:], func=AF.Copy,
                                             bias=8.0)
                        # rhs = [p*xg | p | hi | lo]
                        rhs = wp.tile([P, CPC, RWB], BF, tag="rhs", bufs=4,
                                      name=f"rhs{layer}_{cb}")
                        rb = rhs[:]
                        xb = xg[:, sub * CPC:(sub + 1) * CPC, :]
                        pb = pe[:]
                        nc.vector.tensor_tensor(
                            out=_ap(rb, [rb.ap[0], [RWB, CPC], [CW, HN], [1, CW]]),
                            in0=_ap(xb, [xb.ap[0], [F, CPC], [CW, HN], [1, CW]]),
                            in1=_ap(pb, [pb.ap[0], [HN, CPC], [1, HN], [0, CW]]),
                            op=mul)
                        pc_out = bass.AP(rb.tensor, rb.offset + F,
                                         [list(rb.ap[0]), [RWB, CPC], [1, HN]])
                        nc.scalar.activation(
                            out=pc_out, in_=pb.rearrange("p (c h) -> p c h", h=HN),
                            func=AF.Copy)
                        hi_out = bass.AP(rb.tensor, rb.offset + F + HN,
                                         [list(rb.ap[0]), [RWB, CPC], [1, HN]])
                        nc.scalar.activation(
                            out=hi_out, in_=thi[:].rearrange(
                                "p (c h) -> p c h", h=HN),
                            func=AF.Copy, bias=-8.0)
                        lo_out = bass.AP(rb.tensor, rb.offset + F + 2 * HN,
                                         [list(rb.ap[0]), [RWB, CPC], [1, HN]])
                        nc.vector.tensor_tensor(
                            out=lo_out,
                            in0=al[:].rearrange("p (c h) -> p c h", h=HN),
                            in1=hi_out, op=mybir.AluOpType.subtract)
                        # scatter matmuls, PSUM-accumulated per block segment
                        for c in range(CPC):
                            ci = cb + c
                            blk = int(chunk_block[ci])
                            seg_start = ci == 0 or chunk_block[ci - 1] != blk
                            seg_end = ci == CH - 1 or chunk_block[ci + 1] != blk
                            if seg_start:
                                bp = pp.tile([P, RWB], F32, tag="bp", bufs=2,
                                             name=f"bp{layer}_{ci}")
                            nc.tensor.matmul(
                                out=bp[:], lhsT=qt[:, c, :], rhs=rhs[:, c, :],
                                start=seg_start, stop=seg_end)
                            if seg_end:
                                nc.vector.tensor_add(
                                    out=hacc[:, blk * RW:blk * RW + RWB],
                                    in0=hacc[:, blk * RW:blk * RW + RWB],
                                    in1=bp[:])

            def epilogue(layer, recip_sb, bb_sb):
                HN = cfg.H1 if layer == 1 else 1
                CW = F // HN
                NB = NBLK
                hb = hacc[:]
                # batched across all blocks: sa = hi + lo, eps, den, rec
                sa = wp.tile([P, NB, HN], F32, tag="sa", bufs=1,
                             name=f"sa{layer}")
                nc.vector.tensor_add(
                    out=sa[:],
                    in0=_ap(hb, [hb.ap[0], [RW, NB], [1, HN]], F + HN),
                    in1=_ap(hb, [hb.ap[0], [RW, NB], [1, HN]], F + 2 * HN))
                eps = wp.tile([P, NB, HN], F32, tag="eps", bufs=1,
                              name=f"eps{layer}")
                nc.scalar.activation(out=eps[:], in_=sa[:], func=AF.Exp,
                                     bias=lneps_sb[:, 0:1])
                den = wp.tile([P, NB, HN], F32, tag="den", bufs=1,
                              name=f"den{layer}")
                nc.vector.tensor_add(
                    out=den[:], in0=_ap(hb, [hb.ap[0], [RW, NB], [1, HN]], F),
                    in1=eps[:])
                rec = wp.tile([P, NB, HN], F32, tag="rec", bufs=1,
                              name=f"rec{layer}")
                nc.vector.reciprocal(out=rec[:], in_=den[:])
                # sc[n, b, c] = rec[n, b, head(c)] * recip_att[c]
                sc = wp.tile([P, NB * F], F32, tag="sc", bufs=1,
                             name=f"sc{layer}")
                scb = sc[:]
                rcb = rec[:]
                rpb = recip_sb[:]
                nc.vector.tensor_tensor(
                    out=_ap(scb, [scb.ap[0], [F, NB], [CW, HN], [1, CW]]),
                    in0=_ap(rcb, [rcb.ap[0], [HN, NB], [1, HN], [0, CW]]),
                    in1=_ap(rpb, [rpb.ap[0], [0, NB], [CW, HN], [1, CW]]),
                    op=mul)
                # y = msg * sc + bias
                so2 = stage_o[:].rearrange("p (b f) -> p b f", f=F)
                nc.vector.tensor_tensor(
                    out=so2, in0=_ap(hb, [hb.ap[0], [RW, NB], [1, F]]),
                    in1=sc[:].rearrange("p (b f) -> p b f", f=F), op=mul)
                bbb = bb_sb[:]
                nc.vector.tensor_tensor(
                    out=so2, in0=so2,
                    in1=_ap(bbb, [bbb.ap[0], [0, NB], [1, F]]),
                    op=mybir.AluOpType.add)
                sob = stage_o[:]
                # elu(y) = relu(y) + exp(-relu(-y)) - 1
                # tmp reuse: stage (xl staging, dead) and xTs (dead after L1)
                nc.scalar.activation(out=stage[:], in_=sob, func=AF.Relu,
                                     scale=-1.0)
                nc.scalar.activation(out=xTs_sb[:], in_=stage[:], func=AF.Exp,
                                     scale=-1.0)
                nc.vector.tensor_scalar_max(out=sob, in0=sob, scalar1=0.0)
                nc.vector.tensor_add(out=sob, in0=sob, in1=xTs_sb[:])
                nc.vector.tensor_scalar_add(out=sob, in0=sob, scalar1=-1.0)
                if layer == 1:
                    for b in range(NBLK):
                        trh = pp.tile([P, 512], F32, tag="trp", bufs=2,
                                      name=f"trh{b}")
                        nc.tensor.transpose(out=trh[:, 0:P],
                                            in_=stage_o[:, b * F:(b + 1) * F],
                                            identity=identF_sb[:])
                        nc.scalar.activation(out=hT_sb[:, b * P:(b + 1) * P],
                                             in_=trh[:, 0:P], func=AF.Copy)

            # ---- layer 1 ----
            node_phase(xTs_sb, w1_sb, xr1_sb)
            dma_stage_to(xl1sh)
            nc.gpsimd.collective_compute(
                "AllGather", mybir.AluOpType.bypass,
                replica_groups=[list(range(8))],
                ins=[xl1sh[:]], outs=[xl1full[:]])
            edge_pass(1, xl1full, xr1_sb, k1)
            epilogue(1, recip1_sb, bb1_sb)
            # ---- layer 2 ----
            node_phase(hT_sb, w2_sb, xr2_sb)
            dma_stage_to(xl2sh)
            nc.gpsimd.collective_compute(
                "AllGather", mybir.AluOpType.bypass,
                replica_groups=[list(range(8))],
                ins=[xl2sh[:]], outs=[xl2full[:]])
            edge_pass(2, xl2full, xr2_sb, k2)
            epilogue(2, recip2_sb, bb2_sb)
            oo = out[:].rearrange("(b p) f -> p b f", p=P)
            ii = stage_o[:].rearrange("p (b f) -> p b f", f=F)
            nc.sync.dma_start(out=oo, in_=ii)

    nc.compile()
    return nc


# ---------------------------------------------------------------------------
# public entry point
# ---------------------------------------------------------------------------
_CACHE = {}
LAST_RESULTS = None


def _trace_enabled():
    import os
    return os.environ.get("GAT_TRACE", "") == "1"


def _install_trace_shim():
    """antenv.axon_hooks is absent in this image; recreate it so trace=True
    can capture NTFF profiles through the axon PJRT plugin."""
    import sys, types
    if "antenv.axon_hooks" in sys.modules:
        return
    try:
        mod = types.ModuleType("antenv.axon_hooks")
        mod._hook = None
        mod.set_axon_ntff_profile_hook = lambda h: setattr(mod, "_hook", h)
        mod.get_axon_ntff_profile_hook = lambda: mod._hook
        sys.modules["antenv.axon_hooks"] = mod
        import antenv
        antenv.axon_hooks = mod
        from trn_agent_boot.trn_boot import _ntff_profile_via_ctypes
        mod._hook = _ntff_profile_via_ctypes("/opt/axon/libaxon_pjrt.so")
        import concourse.bass_utils as bu
        bu.upload_artifacts = lambda tmpdir: str(tmpdir)
    except Exception:
        pass


def kernel(x, edge_index, Wl1, Wr1, att1, b1, Wl2, Wr2, att2, b2):
    global LAST_RESULTS
    from concourse.bass_utils import run_bass_kernel_spmd

    trace = _trace_enabled()
    if trace:
        _install_trace_shim()

    x = np.asarray(x, np.float32)
    edge_index = np.asarray(edge_index)
    N, E = x.shape[0], edge_index.shape[1]
    cfg = Cfg(N, E, nblk=49)

    per_core, struct = host_prep(cfg, x, edge_index)
    consts, meta = host_consts(cfg, Wl1, Wr1, att1, b1, Wl2, Wr2, att2, b2)

    key = (N, E, x.shape[1], struct["S_A"], struct["S_B"],
           meta["k1"], meta["k2"])
    if key not in _CACHE:
        _CACHE[key] = build_program(cfg, struct, meta["k1"], meta["k2"])
    nc = _CACHE[key]

    in_maps = []
    for k in range(8):
        m = dict(per_core[k])
        m.update(consts)
        in_maps.append(m)
    res = run_bass_kernel_spmd(nc, in_maps, core_ids=list(range(8)), trace=trace)
    LAST_RESULTS = res
    outs = [np.asarray(res.results[k]["out"]) for k in range(8)]
    full = np.concatenate(outs, axis=0)[:N].astype(np.float32)
    unperm = np.empty_like(full)
    unperm[:, meta["perm2"]] = full
    return unperm


# revision 46
# speedup vs baseline: 1.4235x; 1.1520x over previous
"""Trainium2 Bass kernel for a 2-layer GATv2 aggregator (N=50000, E=800000).

Self-contained: kernel(**inputs) takes full inputs, shards across 8
NeuronCores internally, returns the full (50000, 128) float32 output.

v2 strategy (8-core SPMD, dst-sharded):
- Channels permuted per head (positive-att first) and tables pre-scaled by the
  SIGNED att value: t~_c = att_c*(xl_c + xr_c).  Then
  att_c*leaky(t_c) = Prelu(t~_c; 0.2) for att_c>0 and min(t~, 0.2 t~)
  = Prelu(0.2*t~; 5) for att_c<0, so alpha = plain per-head sum of the
  ACT output -- one strided reduce, no per-edge att multiply.
- Messages aggregate Sum p*x~l (scaled); epilogue divides by att_c per
  channel (recip const tile).  Layer-2 weight rows pre-permuted; final
  output unpermuted on host.
- All edge-pass tiles bf16 (tables, one-hots, rhs);  eps term
  1e-16*exp(sum alpha) (replicates the oracle's segment_max-is-sum bug)
  accumulated via exact hi/lo bf16 split columns in the scatter matmul.
- dma_gather with prepare_only+trigger_dma so SWDGE desc-gen overlaps
  the DMA drain;  gather calls of 2048 edges (bf16 rows, 256 B).
"""
import numpy as np
import ml_dtypes

import concourse.bass as bass
import concourse.bacc as bacc
import concourse.mybir as mybir
from concourse.tile import TileContext

BF16 = ml_dtypes.bfloat16
F32 = mybir.dt.float32
BF = mybir.dt.bfloat16
F8 = mybir.dt.float8e4
I16 = mybir.dt.int16
PAD_DST = 200.0
P = 128
CPC = 8           # chunks per compute group
GPC = 8           # chunks per gather call (1024 idx = SWDGE ring max)
NI = GPC * 128    # indices per gather call
import os
USE_PREP = os.environ.get("GAT_PREP", "0") == "1"


class Cfg:
    def __init__(self, N, E, nblk, feat=128, heads1=2):
        self.N, self.E = N, E
        self.NBLK = nblk
        self.SHARD = nblk * P
        self.NPAD = 8 * self.SHARD
        assert self.NPAD >= N and self.NPAD % 256 == 0
        self.HALF = self.NPAD // 2
        assert self.HALF <= 32767
        self.F = feat
        self.H1 = heads1
        self.C1 = feat // heads1


def host_prep(cfg, x, edge_index):
    """Returns (per_core_inputs: list of dict, struct: dict)."""
    N, E = cfg.N, cfg.E
    src = np.concatenate([np.asarray(edge_index[0]), np.arange(N)]).astype(np.int64)
    dst = np.concatenate([np.asarray(edge_index[1]), np.arange(N)]).astype(np.int64)
    ET = src.shape[0]

    core = dst // cfg.SHARD
    block = (dst % cfg.SHARD) // P
    dloc = dst % P
    half = (src >= cfg.HALF).astype(np.int64)
    gval = (src - half * cfg.HALF).astype(np.int64)

    # group = (core, half, block); rank within group
    key = (core * 2 + half) * cfg.NBLK + block
    order = np.argsort(key, kind="stable")
    key_s = key[order]
    ngroups = 8 * 2 * cfg.NBLK
    cnt = np.bincount(key_s, minlength=ngroups)
    starts = np.zeros(ngroups + 1, np.int64)
    np.cumsum(cnt, out=starts[1:])
    rank = np.arange(ET) - starts[key_s]

    cnt3 = cnt.reshape(8, 2, cfg.NBLK)
    S_A = int(np.ceil(cnt3[:, 0, :].max() / P))
    S_B = int(np.ceil(cnt3[:, 1, :].max() / P))
    S_A, S_B = max(S_A, 1), max(S_B, 1)
    CHA = -(-(cfg.NBLK * S_A) // GPC) * GPC
    CHB = -(-(cfg.NBLK * S_B) // GPC) * GPC
    CH = CHA + CHB
    CALLS = CH // GPC

    chunk_half = np.zeros(CH, np.int64)
    chunk_block = np.zeros(CH, np.int64)
    for c in range(CH):
        if c < CHA:
            chunk_half[c] = 0
            chunk_block[c] = min(c // S_A, cfg.NBLK - 1)
        else:
            chunk_half[c] = 1
            chunk_block[c] = min((c - CHA) // S_B, cfg.NBLK - 1)

    gidx = np.zeros((8, CH, P), np.int16)
    dstl = np.full((8, CH, P), PAD_DST, np.float32)
    g_half = half[order]
    g_core = core[order]
    g_block = block[order]
    slot_base = np.where(g_half == 0, g_block * S_A, CHA + g_block * S_B)
    slot = slot_base + rank // P
    pos = rank % P
    gidx[g_core, slot, pos] = gval[order].astype(np.int16)
    dstl[g_core, slot, pos] = dloc[order].astype(np.float32)

    # host-built scatter one-hots qt[e, n] = (dstl == n), streamed to SBUF by
    # HWDGE DMA (keeps the build off DVE, which serializes with SWDGE gathers);
    # fp8 to halve the stream's SDMA traffic (0.0/1.0 are exact)
    F8NP = ml_dtypes.float8_e4m3
    qts = np.zeros((8, P, CH * P), F8NP)
    narr = np.arange(P, dtype=np.float32)
    for k in range(8):
        oh = (dstl[k][:, :, None] == narr[None, None, :])  # [CH, e, n]
        qts[k] = np.ascontiguousarray(
            oh.transpose(1, 0, 2).reshape(P, CH * P)).astype(F8NP)

    # wrap gather indices per call of NI: [NI//16,16].T -> [16, NI//16]
    gw = gidx.reshape(8, CALLS, NI // 16, 16).transpose(0, 1, 3, 2)
    gw = gw.transpose(0, 2, 1, 3).reshape(8, 16, CALLS * (NI // 16))
    gw = np.tile(gw, (1, 8, 1))  # replicate to 128 partitions

    struct = dict(S_A=S_A, S_B=S_B, CHA=CHA, CHB=CHB, CH=CH, CALLS=CALLS,
                  chunk_half=chunk_half, chunk_block=chunk_block)

    x_pad = np.zeros((cfg.NPAD, cfg.F), np.float32)
    x_pad[:N] = np.asarray(x, np.float32)

    per_core = []
    for k in range(8):
        per_core.append(dict(
            xTs=np.ascontiguousarray(
                x_pad[k * cfg.SHARD:(k + 1) * cfg.SHARD].T.astype(BF16)),
            gidx=np.ascontiguousarray(gw[k]),
            qts=qts[k],
        ))
    return per_core, struct


def _perm_layer(Wl, Wr, att):
    """Channel perm (positive att first per head) + signed-scale weights."""
    att = np.asarray(att, np.float32)
    H, C = att.shape
    perm = np.zeros((H, C), np.int64)
    k = np.zeros(H, np.int64)
    for h in range(H):
        pos = np.where(att[h] > 0)[0]
        neg = np.where(att[h] <= 0)[0]
        perm[h] = np.concatenate([pos, neg])
        k[h] = len(pos)
    att_p = np.take_along_axis(att, perm, axis=1)
    s = att_p.reshape(-1)                    # signed scale per (permuted) chan
    flat_perm = (perm + np.arange(H)[:, None] * C).reshape(-1)
    Wl_t = np.asarray(Wl, np.float32)[:, flat_perm] * s[None, :]
    Wr_t = np.asarray(Wr, np.float32)[:, flat_perm] * s[None, :]
    return Wl_t, Wr_t, s, k, flat_perm


def host_consts(cfg, Wl1, Wr1, att1, b1, Wl2, Wr2, att2, b2):
    f = cfg.F
    Wl1t, Wr1t, s1, k1, perm1 = _perm_layer(Wl1, Wr1, att1)
    # layer2 rows permuted by perm1 (its input h is in permuted-1 order)
    Wl2t, Wr2t, s2, k2, perm2 = _perm_layer(
        np.asarray(Wl2, np.float32)[perm1], np.asarray(Wr2, np.float32)[perm1],
        att2)
    c = {}
    c["w1"] = np.hstack([Wl1t, Wr1t]).astype(BF16)
    c["w2"] = np.hstack([Wl2t, Wr2t]).astype(BF16)
    c["recip1"] = np.tile((1.0 / s1).reshape(1, f), (P, 1)).astype(np.float32)
    c["recip2"] = np.tile((1.0 / s2).reshape(1, f), (P, 1)).astype(np.float32)
    c["bb1"] = np.tile(np.asarray(b1, np.float32)[perm1].reshape(1, f), (P, 1))
    c["bb2"] = np.tile(np.asarray(b2, np.float32)[perm2].reshape(1, f), (P, 1))
    c["identB"] = np.eye(P, dtype=np.float32).astype(BF16)
    c["identB8"] = np.eye(P, dtype=np.float32).astype(ml_dtypes.float8_e4m3)
    c["identF"] = np.eye(P, dtype=np.float32)
    meta = dict(k1=tuple(int(v) for v in k1), k2=tuple(int(v) for v in k2),
                perm2=perm2)
    return c, meta


def _ap(base, layout, extra_offset=0):
    return bass.AP(base.tensor, base.offset + extra_offset,
                   [list(d) for d in layout])


def build_program(cfg, struct, k1, k2):
    NBLK, SHARD, NPAD, HALF, F = cfg.NBLK, cfg.SHARD, cfg.NPAD, cfg.HALF, cfg.F
    CH, CALLS = struct["CH"], struct["CALLS"]
    chunk_half, chunk_block = struct["chunk_half"], struct["chunk_block"]
    RW = F + 8

    nc = bacc.Bacc("TRN2", target_bir_lowering=False, debug=False,
                   num_devices=8, num_swdge_queues=4)

    xTs = nc.dram_tensor("xTs", [P, SHARD], BF, kind="ExternalInput")
    gidx = nc.dram_tensor("gidx", [P, CALLS * (NI // 16)], I16, kind="ExternalInput")
    qts = nc.dram_tensor("qts", [P, CH * P], F8, kind="ExternalInput")
    w1 = nc.dram_tensor("w1", [P, 2 * F], BF, kind="ExternalInput")
    w2 = nc.dram_tensor("w2", [P, 2 * F], BF, kind="ExternalInput")
    recip1 = nc.dram_tensor("recip1", [P, F], F32, kind="ExternalInput")
    recip2 = nc.dram_tensor("recip2", [P, F], F32, kind="ExternalInput")
    bb1 = nc.dram_tensor("bb1", [P, F], F32, kind="ExternalInput")
    bb2 = nc.dram_tensor("bb2", [P, F], F32, kind="ExternalInput")
    identB = nc.dram_tensor("identB", [P, P], BF, kind="ExternalInput")
    identB8 = nc.dram_tensor("identB8", [P, P], F8, kind="ExternalInput")
    identF = nc.dram_tensor("identF", [P, P], F32, kind="ExternalInput")
    out = nc.dram_tensor("out", [SHARD, F], F32, kind="ExternalOutput")

    eq = mybir.AluOpType.is_equal
    mul = mybir.AluOpType.mult
    AF = mybir.ActivationFunctionType
    AX = mybir.AxisListType.X

    with TileContext(nc) as tc:
        with (
            tc.tile_pool(name="const", bufs=1) as cpool,
            tc.tile_pool(name="big", bufs=1) as bigp,
            tc.tile_pool(name="work", bufs=1) as wp,
            tc.tile_pool(name="psum", bufs=1, space="PSUM") as pp,
            tc.tile_pool(name="dram", bufs=1, space="DRAM") as dp,
        ):
            def load_const(t, shape, dt):
                s = cpool.tile(shape, dt, name=t.name + "_sb")
                nc.sync.dma_start(out=s[:], in_=t[:])
                return s
            w1_sb = load_const(w1, [P, 2 * F], BF)
            w2_sb = load_const(w2, [P, 2 * F], BF)
            recip1_sb = load_const(recip1, [P, F], F32)
            recip2_sb = load_const(recip2, [P, F], F32)
            bb1_sb = load_const(bb1, [P, F], F32)
            bb2_sb = load_const(bb2, [P, F], F32)
            identB_sb = load_const(identB, [P, P], BF)
            identB8_sb = load_const(identB8, [P, P], F8)
            identF_sb = load_const(identF, [P, P], F32)
            xTs_sb = bigp.tile([P, SHARD], BF, name="xTs_sb")
            nc.sync.dma_start(out=xTs_sb[:], in_=xTs[:])
            gidx_sb = bigp.tile([P, CALLS * (NI // 16)], I16, name="gidx_sb")
            nc.sync.dma_start(out=gidx_sb[:], in_=gidx[:])

            lneps_sb = cpool.tile([P, 1], F32, name="lneps_sb")
            nc.vector.memset(lneps_sb[:], float(np.log(1e-16)))
            xr1_sb = bigp.tile([P, SHARD], BF, name="xr1_sb")
            xr2_sb = bigp.tile([P, SHARD], BF, name="xr2_sb")
            hT_sb = bigp.tile([P, SHARD], BF, name="hT_sb")
            hacc = bigp.tile([P, NBLK * RW], F32, name="hacc")
            stage = bigp.tile([P, SHARD], BF, name="stage")      # xl (bf16)
            stage_o = bigp.tile([P, SHARD], F32, name="stage_o")  # epilogue f32

            xl1sh = dp.tile([SHARD, F], BF, name="xl1sh")
            xl1full = dp.tile([NPAD, F], BF, name="xl1full", addr_space="Shared")
            xl2sh = dp.tile([SHARD, F], BF, name="xl2sh")
            xl2full = dp.tile([NPAD, F], BF, name="xl2full", addr_space="Shared")

            dma_sem = nc.alloc_semaphore("gat_dma")

            def node_phase(src_sb, w_sb, xr_dst):
                for j in range(NBLK):
                    mm = pp.tile([P, 8, P], F32, tag="txr", bufs=2, name=f"mm{j}")
                    mf = mm[:].rearrange("p c f -> p (c f)")
                    nc.tensor.matmul(out=mf[:, 0:2 * F],
                                     lhsT=src_sb[:, j * P:(j + 1) * P],
                                     rhs=w_sb[:], start=True, stop=True)
                    nc.scalar.activation(out=stage[:, j * F:(j + 1) * F],
                                         in_=mf[:, 0:F], func=AF.Copy)
                    nc.scalar.activation(out=xr_dst[:, j * P:(j + 1) * P],
                                         in_=mf[:, F:2 * F], func=AF.Copy)

            def dma_stage_to(dram_tile):
                o = dram_tile[:].rearrange("(b p) f -> p b f", p=P)
                i = stage[:].rearrange("p (b f) -> p b f", f=F)
                nc.sync.dma_start(out=o, in_=i)

            def edge_pass(layer, table, xr_sb, ks):
                HN = cfg.H1 if layer == 1 else 1
                CW = F // HN
                RWB = F + 3 * HN
                nc.vector.memset(hacc[:], 0.0)
                bp = None
                for g in range(CALLS):
                    cb0 = g * GPC
                    hf = int(chunk_half[cb0])
                    tab = table[:][0:HALF, :] if hf == 0 else table[:][HALF:NPAD, :]
                    xg = wp.tile([P, GPC, F], BF, tag="xg", bufs=6,
                                 name=f"xg{layer}_{g}")
                    if USE_PREP:
                        nc.gpsimd.dma_gather(
                            out_ap=xg[:], in_ap=tab,
                            idxs_ap=gidx_sb[:, g * (NI // 16):(g + 1) * (NI // 16)],
                            num_idxs=NI, num_idxs_reg=NI, elem_size=F,
                            prepare_only=True, sem=dma_sem, queue_num=0)
                        nc.gpsimd.trigger_dma(count=None)
                    else:
                        nc.gpsimd.dma_gather(
                            out_ap=xg[:], in_ap=tab,
                            idxs_ap=gidx_sb[:, g * (NI // 16):(g + 1) * (NI // 16)],
                            num_idxs=NI, num_idxs_reg=NI, elem_size=F,
                            queue_num=g % 4)
                    for sub in range(GPC // CPC):
                        cb = cb0 + sub * CPC
                        # one-hot QT[e, n] (lhsT for scatter) streamed from host
                        qt = wp.tile([P, CPC, P], F8, tag="qt", bufs=6,
                                     name=f"qt{layer}_{cb}")
                        nc.sync.dma_start(
                            out=qt[:],
                            in_=qts[:, cb * P:(cb + CPC) * P].rearrange(
                                "p (c f) -> p c f", f=P))
                        # Q[n, e] one-hot = PE transpose of QT; copy to SBUF on ACT
                        # fp8 transpose writes with element step 2 (16-bit
                        # granularity); tile holds 2*CPC*P fp8 = 2048 B
                        trp = pp.tile([P, 2 * CPC * P], F8, tag="trp", bufs=2,
                                      name=f"trp{layer}_{cb}")
                        tpb = trp[:]
                        for c in range(CPC):
                            nc.tensor.transpose(
                                out=_ap(tpb, [tpb.ap[0], [2, P]], 2 * c * P),
                                in_=qt[:, c, :], identity=identB8_sb[:])
                        q = wp.tile([P, CPC, P], F8, tag="q", bufs=6,
                                    name=f"q{layer}_{cb}")
                        nc.scalar.activation(
                            out=q[:],
                            in_=_ap(tpb, [tpb.ap[0], [2 * P, CPC], [2, P]]),
                            func=AF.Copy)
                        # t~ = Q.T @ xr (+ xg via identity matmul), PSUM f32
                        txr = pp.tile([P, CPC, P], F32, tag="txr", bufs=2,
                                      name=f"txr{layer}_{cb}")
                        for c in range(CPC):
                            blk = int(chunk_block[cb + c])
                            nc.tensor.matmul(
                                out=txr[:, c, :], lhsT=q[:, c, :],
                                rhs=xr_sb[:, blk * P:(blk + 1) * P],
                                start=True, stop=False)
                            nc.tensor.matmul(
                                out=txr[:, c, :], lhsT=identB_sb[:],
                                rhs=xg[:, sub * CPC + c, :],
                                start=False, stop=True)
                        # att_c*leaky(t_c): Prelu(x;.2) pos block,
                        # Prelu(.2x;5)=min(x,.2x) neg block, per head
                        lr = wp.tile([P, CPC, F], BF, tag="lr", bufs=4,
                                     name=f"lr{layer}_{cb}")
                        tb = txr[:]
                        lb = lr[:]
                        for h in range(HN):
                            kh = ks[h]
                            if kh > 0:
                                ap_i = bass.AP(tb.tensor, tb.offset + h * CW,
                                               [list(tb.ap[0]), [F, CPC], [1, kh]])
                                ap_o = bass.AP(lb.tensor, lb.offset + h * CW,
                                               [list(lb.ap[0]), [F, CPC], [1, kh]])
                                nc.scalar.activation(out=ap_o, in_=ap_i,
                                                     func=AF.Prelu, alpha=0.2)
                            if kh < CW:
                                off = h * CW + kh
                                ap_i = bass.AP(tb.tensor, tb.offset + off,
                                               [list(tb.ap[0]), [F, CPC],
                                                [1, CW - kh]])
                                ap_o = bass.AP(lb.tensor, lb.offset + off,
                                               [list(lb.ap[0]), [F, CPC],
                                                [1, CW - kh]])
                                nc.scalar.activation(out=ap_o, in_=ap_i,
                                                     func=AF.Prelu, alpha=5.0,
                                                     scale=0.2)
                        # alpha = per-head sum
                        al = wp.tile([P, CPC * HN], F32, tag="al", bufs=4,
                                     name=f"al{layer}_{cb}")
                        nc.vector.reduce_sum(
                            out=al[:],
                            in_=lr[:].rearrange("p c (h s) -> p c h s", s=CW),
                            axis=AX)
                        # p = exp(alpha) (bf16)
                        pe = wp.tile([P, CPC * HN], BF, tag="pe", bufs=4,
                                     name=f"pe{layer}_{cb}")
                        nc.scalar.activation(out=pe[:], in_=al[:], func=AF.Exp)
                        # thi = bf16(al + 8): snaps al to the exact bf16 grid
                        thi = wp.tile([P, CPC * HN], BF, tag="thi", bufs=4,
                                      name=f"thi{layer}_{cb}")
                        nc.scalar.activation(out=thi[:], in_=al[:], func=AF.Copy,
                                             bias=8.0)
                        # rhs = [p*xg | p | hi | lo]
                        rhs = wp.tile([P, CPC, RWB], BF, tag="rhs", bufs=6,
                                      name=f"rhs{layer}_{cb}")
                        rb = rhs[:]
                        xb = xg[:, sub * CPC:(sub + 1) * CPC, :]
                        pb = pe[:]
                        nc.vector.tensor_tensor(
                            out=_ap(rb, [rb.ap[0], [RWB, CPC], [CW, HN], [1, CW]]),
                            in0=_ap(xb, [xb.ap[0], [F, CPC], [CW, HN], [1, CW]]),
                            in1=_ap(pb, [pb.ap[0], [HN, CPC], [1, HN], [0, CW]]),
                            op=mul)
                        pc_out = bass.AP(rb.tensor, rb.offset + F,
                                         [list(rb.ap[0]), [RWB, CPC], [1, HN]])
                        nc.scalar.activation(
                            out=pc_out, in_=pb.rearrange("p (c h) -> p c h", h=HN),
                            func=AF.Copy)
                        hi_out = bass.AP(rb.tensor, rb.offset + F + HN,
                                         [list(rb.ap[0]), [RWB, CPC], [1, HN]])
                        nc.scalar.activation(
                            out=hi_out, in_=thi[:].rearrange(
                                "p (c h) -> p c h", h=HN),
                            func=AF.Copy, bias=-8.0)
                        lo_out = bass.AP(rb.tensor, rb.offset + F + 2 * HN,
                                         [list(rb.ap[0]), [RWB, CPC], [1, HN]])
                        nc.vector.tensor_tensor(
                            out=lo_out,
                            in0=al[:].rearrange("p (c h) -> p c h", h=HN),
                            in1=hi_out, op=mybir.AluOpType.subtract)
                        # scatter matmuls, PSUM-accumulated per block segment
                        for c in range(CPC):
                            ci = cb + c
                            blk = int(chunk_block[ci])
                            seg_start = ci == 0 or chunk_block[ci - 1] != blk
                            seg_end = ci == CH - 1 or chunk_block[ci + 1] != blk
                            if seg_start:
                                bp = pp.tile([P, RWB], F32, tag="bp", bufs=2,
                                             name=f"bp{layer}_{ci}")
                            nc.tensor.matmul(
                                out=bp[:], lhsT=qt[:, c, :], rhs=rhs[:, c, :],
                                start=seg_start, stop=seg_end)
                            if seg_end:
                                nc.vector.tensor_add(
                                    out=hacc[:, blk * RW:blk * RW + RWB],
                                    in0=hacc[:, blk * RW:blk * RW + RWB],
                                    in1=bp[:])

            def epilogue(layer, recip_sb, bb_sb):
                HN = cfg.H1 if layer == 1 else 1
                CW = F // HN
                NB = NBLK
                hb = hacc[:]
                # batched across all blocks: sa = hi + lo, eps, den, rec
                sa = wp.tile([P, NB, HN], F32, tag="sa", bufs=1,
                             name=f"sa{layer}")
                nc.vector.tensor_add(
                    out=sa[:],
                    in0=_ap(hb, [hb.ap[0], [RW, NB], [1, HN]], F + HN),
                    in1=_ap(hb, [hb.ap[0], [RW, NB], [1, HN]], F + 2 * HN))
                eps = wp.tile([P, NB, HN], F32, tag="eps", bufs=1,
                              name=f"eps{layer}")
                nc.scalar.activation(out=eps[:], in_=sa[:], func=AF.Exp,
                                     bias=lneps_sb[:, 0:1])
                den = wp.tile([P, NB, HN], F32, tag="den", bufs=1,
                              name=f"den{layer}")
                nc.vector.tensor_add(
                    out=den[:], in0=_ap(hb, [hb.ap[0], [RW, NB], [1, HN]], F),
                    in1=eps[:])
                rec = wp.tile([P, NB, HN], F32, tag="rec", bufs=1,
                              name=f"rec{layer}")
                nc.vector.reciprocal(out=rec[:], in_=den[:])
                # sc[n, b, c] = rec[n, b, head(c)] * recip_att[c]
                sc = wp.tile([P, NB * F], F32, tag="sc", bufs=1,
                             name=f"sc{layer}")
                scb = sc[:]
                rcb = rec[:]
                rpb = recip_sb[:]
                nc.vector.tensor_tensor(
                    out=_ap(scb, [scb.ap[0], [F, NB], [CW, HN], [1, CW]]),
                    in0=_ap(rcb, [rcb.ap[0], [HN, NB], [1, HN], [0, CW]]),
                    in1=_ap(rpb, [rpb.ap[0], [0, NB], [CW, HN], [1, CW]]),
                    op=mul)
                # y = msg * sc + bias
                so2 = stage_o[:].rearrange("p (b f) -> p b f", f=F)
                nc.vector.tensor_tensor(
                    out=so2, in0=_ap(hb, [hb.ap[0], [RW, NB], [1, F]]),
                    in1=sc[:].rearrange("p (b f) -> p b f", f=F), op=mul)
                bbb = bb_sb[:]
                nc.vector.tensor_tensor(
                    out=so2, in0=so2,
                    in1=_ap(bbb, [bbb.ap[0], [0, NB], [1, F]]),
                    op=mybir.AluOpType.add)
                sob = stage_o[:]
                # elu(y) = relu(y) + exp(-relu(-y)) - 1
                # tmp reuse: stage (xl staging, dead) and xTs (dead after L1)
                nc.scalar.activation(out=stage[:], in_=sob, func=AF.Relu,
                                     scale=-1.0)
                nc.scalar.activation(out=xTs_sb[:], in_=stage[:], func=AF.Exp,
                                     scale=-1.0)
                nc.vector.tensor_scalar_max(out=sob, in0=sob, scalar1=0.0)
                nc.vector.tensor_add(out=sob, in0=sob, in1=xTs_sb[:])
                nc.vector.tensor_scalar_add(out=sob, in0=sob, scalar1=-1.0)
                if layer == 1:
                    for b in range(NBLK):
                        trh = pp.tile([P, 512], F32, tag="trp", bufs=2,
                                      name=f"trh{b}")
                        nc.tensor.transpose(out=trh[:, 0:P],
                                            in_=stage_o[:, b * F:(b + 1) * F],
                                            identity=identF_sb[:])
                        nc.scalar.activation(out=hT_sb[:, b * P:(b + 1) * P],
                                             in_=trh[:, 0:P], func=AF.Copy)

            # ---- layer 1 ----
            node_phase(xTs_sb, w1_sb, xr1_sb)
            dma_stage_to(xl1sh)
            nc.gpsimd.collective_compute(
                "AllGather", mybir.AluOpType.bypass,
                replica_groups=[list(range(8))],
                ins=[xl1sh[:]], outs=[xl1full[:]])
            edge_pass(1, xl1full, xr1_sb, k1)
            epilogue(1, recip1_sb, bb1_sb)
            # ---- layer 2 ----
            node_phase(hT_sb, w2_sb, xr2_sb)
            dma_stage_to(xl2sh)
            nc.gpsimd.collective_compute(
                "AllGather", mybir.AluOpType.bypass,
                replica_groups=[list(range(8))],
                ins=[xl2sh[:]], outs=[xl2full[:]])
            edge_pass(2, xl2full, xr2_sb, k2)
            epilogue(2, recip2_sb, bb2_sb)
            oo = out[:].rearrange("(b p) f -> p b f", p=P)
            ii = stage_o[:].rearrange("p (b f) -> p b f", f=F)
            nc.sync.dma_start(out=oo, in_=ii)

    nc.compile()
    return nc


# ---------------------------------------------------------------------------
# public entry point
# ---------------------------------------------------------------------------
_CACHE = {}
LAST_RESULTS = None


def _trace_enabled():
    import os
    return os.environ.get("GAT_TRACE", "") == "1"


def _install_trace_shim():
    """antenv.axon_hooks is absent in this image; recreate it so trace=True
    can capture NTFF profiles through the axon PJRT plugin."""
    import sys, types
    if "antenv.axon_hooks" in sys.modules:
        return
    try:
        mod = types.ModuleType("antenv.axon_hooks")
        mod._hook = None
        mod.set_axon_ntff_profile_hook = lambda h: setattr(mod, "_hook", h)
        mod.get_axon_ntff_profile_hook = lambda: mod._hook
        sys.modules["antenv.axon_hooks"] = mod
        import antenv
        antenv.axon_hooks = mod
        from trn_agent_boot.trn_boot import _ntff_profile_via_ctypes
        mod._hook = _ntff_profile_via_ctypes("/opt/axon/libaxon_pjrt.so")
        import concourse.bass_utils as bu
        bu.upload_artifacts = lambda tmpdir: str(tmpdir)
    except Exception:
        pass


def kernel(x, edge_index, Wl1, Wr1, att1, b1, Wl2, Wr2, att2, b2):
    global LAST_RESULTS
    from concourse.bass_utils import run_bass_kernel_spmd

    trace = _trace_enabled()
    if trace:
        _install_trace_shim()

    x = np.asarray(x, np.float32)
    edge_index = np.asarray(edge_index)
    N, E = x.shape[0], edge_index.shape[1]
    cfg = Cfg(N, E, nblk=49)

    per_core, struct = host_prep(cfg, x, edge_index)
    consts, meta = host_consts(cfg, Wl1, Wr1, att1, b1, Wl2, Wr2, att2, b2)

    key = (N, E, x.shape[1], struct["S_A"], struct["S_B"],
           meta["k1"], meta["k2"])
    if key not in _CACHE:
        _CACHE[key] = build_program(cfg, struct, meta["k1"], meta["k2"])
    nc = _CACHE[key]

    in_maps = []
    for k in range(8):
        m = dict(per_core[k])
        m.update(consts)
        in_maps.append(m)
    res = run_bass_kernel_spmd(nc, in_maps, core_ids=list(range(8)), trace=trace)
    LAST_RESULTS = res
    outs = [np.asarray(res.results[k]["out"]) for k in range(8)]
    full = np.concatenate(outs, axis=0)[:N].astype(np.float32)
    unperm = np.empty_like(full)
    unperm[:, meta["perm2"]] = full
    return unperm


# revision 48
# speedup vs baseline: 1.5087x; 1.0598x over previous
"""Trainium2 Bass kernel for a 2-layer GATv2 aggregator (N=50000, E=800000).

Self-contained: kernel(**inputs) takes full inputs, shards across 8
NeuronCores internally, returns the full (50000, 128) float32 output.

v2 strategy (8-core SPMD, dst-sharded):
- Channels permuted per head (positive-att first) and tables pre-scaled by the
  SIGNED att value: t~_c = att_c*(xl_c + xr_c).  Then
  att_c*leaky(t_c) = Prelu(t~_c; 0.2) for att_c>0 and min(t~, 0.2 t~)
  = Prelu(0.2*t~; 5) for att_c<0, so alpha = plain per-head sum of the
  ACT output -- one strided reduce, no per-edge att multiply.
- Messages aggregate Sum p*x~l (scaled); epilogue divides by att_c per
  channel (recip const tile).  Layer-2 weight rows pre-permuted; final
  output unpermuted on host.
- All edge-pass tiles bf16 (tables, one-hots, rhs);  eps term
  1e-16*exp(sum alpha) (replicates the oracle's segment_max-is-sum bug)
  accumulated via exact hi/lo bf16 split columns in the scatter matmul.
- dma_gather with prepare_only+trigger_dma so SWDGE desc-gen overlaps
  the DMA drain;  gather calls of 2048 edges (bf16 rows, 256 B).
"""
import numpy as np
import ml_dtypes

import concourse.bass as bass
import concourse.bacc as bacc
import concourse.mybir as mybir
from concourse.tile import TileContext

BF16 = ml_dtypes.bfloat16
F32 = mybir.dt.float32
BF = mybir.dt.bfloat16
F8 = mybir.dt.float8e4
I16 = mybir.dt.int16
PAD_DST = 200.0
P = 128
CPC = 8           # chunks per compute group
GPC = 8           # chunks per gather call (1024 idx = SWDGE ring max)
NI = GPC * 128    # indices per gather call
import os
USE_PREP = os.environ.get("GAT_PREP", "0") == "1"


class Cfg:
    def __init__(self, N, E, nblk, feat=128, heads1=2):
        self.N, self.E = N, E
        self.NBLK = nblk
        self.SHARD = nblk * P
        self.NPAD = 8 * self.SHARD
        assert self.NPAD >= N and self.NPAD % 256 == 0
        self.HALF = self.NPAD // 2
        assert self.HALF <= 32767
        self.F = feat
        self.H1 = heads1
        self.C1 = feat // heads1


def host_prep(cfg, x, edge_index):
    """Returns (per_core_inputs: list of dict, struct: dict)."""
    N, E = cfg.N, cfg.E
    src = np.concatenate([np.asarray(edge_index[0]), np.arange(N)]).astype(np.int64)
    dst = np.concatenate([np.asarray(edge_index[1]), np.arange(N)]).astype(np.int64)
    ET = src.shape[0]

    core = dst // cfg.SHARD
    block = (dst % cfg.SHARD) // P
    dloc = dst % P
    half = (src >= cfg.HALF).astype(np.int64)
    gval = (src - half * cfg.HALF).astype(np.int64)

    # group = (core, half, block); rank within group
    key = (core * 2 + half) * cfg.NBLK + block
    order = np.argsort(key, kind="stable")
    key_s = key[order]
    ngroups = 8 * 2 * cfg.NBLK
    cnt = np.bincount(key_s, minlength=ngroups)
    starts = np.zeros(ngroups + 1, np.int64)
    np.cumsum(cnt, out=starts[1:])
    rank = np.arange(ET) - starts[key_s]

    cnt3 = cnt.reshape(8, 2, cfg.NBLK)
    S_A = int(np.ceil(cnt3[:, 0, :].max() / P))
    S_B = int(np.ceil(cnt3[:, 1, :].max() / P))
    S_A, S_B = max(S_A, 1), max(S_B, 1)
    CHA = -(-(cfg.NBLK * S_A) // GPC) * GPC
    CHB = -(-(cfg.NBLK * S_B) // GPC) * GPC
    CH = CHA + CHB
    CALLS = CH // GPC

    chunk_half = np.zeros(CH, np.int64)
    chunk_block = np.zeros(CH, np.int64)
    for c in range(CH):
        if c < CHA:
            chunk_half[c] = 0
            chunk_block[c] = min(c // S_A, cfg.NBLK - 1)
        else:
            chunk_half[c] = 1
            chunk_block[c] = min((c - CHA) // S_B, cfg.NBLK - 1)

    gidx = np.zeros((8, CH, P), np.int16)
    dstl = np.full((8, CH, P), PAD_DST, np.float32)
    g_half = half[order]
    g_core = core[order]
    g_block = block[order]
    slot_base = np.where(g_half == 0, g_block * S_A, CHA + g_block * S_B)
    slot = slot_base + rank // P
    pos = rank % P
    gidx[g_core, slot, pos] = gval[order].astype(np.int16)
    dstl[g_core, slot, pos] = dloc[order].astype(np.float32)

    # host-built scatter one-hots qt[e, n] = (dstl == n), streamed to SBUF by
    # HWDGE DMA (keeps the build off DVE, which serializes with SWDGE gathers);
    # fp8 to halve the stream's SDMA traffic (0.0/1.0 are exact)
    F8NP = ml_dtypes.float8_e4m3
    qts = np.zeros((8, P, CH * P), F8NP)
    narr = np.arange(P, dtype=np.float32)
    for k in range(8):
        oh = (dstl[k][:, :, None] == narr[None, None, :])  # [CH, e, n]
        qts[k] = np.ascontiguousarray(
            oh.transpose(1, 0, 2).reshape(P, CH * P)).astype(F8NP)

    # wrap gather indices per call of NI: [NI//16,16].T -> [16, NI//16]
    gw = gidx.reshape(8, CALLS, NI // 16, 16).transpose(0, 1, 3, 2)
    gw = gw.transpose(0, 2, 1, 3).reshape(8, 16, CALLS * (NI // 16))
    gw = np.tile(gw, (1, 8, 1))  # replicate to 128 partitions

    struct = dict(S_A=S_A, S_B=S_B, CHA=CHA, CHB=CHB, CH=CH, CALLS=CALLS,
                  chunk_half=chunk_half, chunk_block=chunk_block)

    x_pad = np.zeros((cfg.NPAD, cfg.F), np.float32)
    x_pad[:N] = np.asarray(x, np.float32)

    per_core = []
    for k in range(8):
        per_core.append(dict(
            xTs=np.ascontiguousarray(
                x_pad[k * cfg.SHARD:(k + 1) * cfg.SHARD].T.astype(BF16)),
            gidx=np.ascontiguousarray(gw[k]),
            qts=qts[k],
        ))
    return per_core, struct


def _perm_layer(Wl, Wr, att):
    """Channel perm (positive att first per head) + signed-scale weights."""
    att = np.asarray(att, np.float32)
    H, C = att.shape
    perm = np.zeros((H, C), np.int64)
    k = np.zeros(H, np.int64)
    for h in range(H):
        pos = np.where(att[h] > 0)[0]
        neg = np.where(att[h] <= 0)[0]
        perm[h] = np.concatenate([pos, neg])
        k[h] = len(pos)
    att_p = np.take_along_axis(att, perm, axis=1)
    s = att_p.reshape(-1)                    # signed scale per (permuted) chan
    flat_perm = (perm + np.arange(H)[:, None] * C).reshape(-1)
    Wl_t = np.asarray(Wl, np.float32)[:, flat_perm] * s[None, :]
    Wr_t = np.asarray(Wr, np.float32)[:, flat_perm] * s[None, :]
    return Wl_t, Wr_t, s, k, flat_perm


def host_consts(cfg, Wl1, Wr1, att1, b1, Wl2, Wr2, att2, b2):
    f = cfg.F
    Wl1t, Wr1t, s1, k1, perm1 = _perm_layer(Wl1, Wr1, att1)
    # layer2 rows permuted by perm1 (its input h is in permuted-1 order)
    Wl2t, Wr2t, s2, k2, perm2 = _perm_layer(
        np.asarray(Wl2, np.float32)[perm1], np.asarray(Wr2, np.float32)[perm1],
        att2)
    c = {}
    c["w1"] = np.hstack([Wl1t, Wr1t]).astype(BF16)
    c["w2"] = np.hstack([Wl2t, Wr2t]).astype(BF16)
    c["recip1"] = np.tile((1.0 / s1).reshape(1, f), (P, 1)).astype(np.float32)
    c["recip2"] = np.tile((1.0 / s2).reshape(1, f), (P, 1)).astype(np.float32)
    c["bb1"] = np.tile(np.asarray(b1, np.float32)[perm1].reshape(1, f), (P, 1))
    c["bb2"] = np.tile(np.asarray(b2, np.float32)[perm2].reshape(1, f), (P, 1))
    c["identB"] = np.eye(P, dtype=np.float32).astype(BF16)
    c["identB8"] = np.eye(P, dtype=np.float32).astype(ml_dtypes.float8_e4m3)
    c["identF"] = np.eye(P, dtype=np.float32)
    meta = dict(k1=tuple(int(v) for v in k1), k2=tuple(int(v) for v in k2),
                perm2=perm2)
    return c, meta


def _ap(base, layout, extra_offset=0):
    return bass.AP(base.tensor, base.offset + extra_offset,
                   [list(d) for d in layout])


def build_program(cfg, struct, k1, k2):
    NBLK, SHARD, NPAD, HALF, F = cfg.NBLK, cfg.SHARD, cfg.NPAD, cfg.HALF, cfg.F
    CH, CALLS = struct["CH"], struct["CALLS"]
    chunk_half, chunk_block = struct["chunk_half"], struct["chunk_block"]
    RW = F + 8

    nc = bacc.Bacc("TRN2", target_bir_lowering=False, debug=False,
                   num_devices=8, num_swdge_queues=4)

    xTs = nc.dram_tensor("xTs", [P, SHARD], BF, kind="ExternalInput")
    gidx = nc.dram_tensor("gidx", [P, CALLS * (NI // 16)], I16, kind="ExternalInput")
    qts = nc.dram_tensor("qts", [P, CH * P], F8, kind="ExternalInput")
    w1 = nc.dram_tensor("w1", [P, 2 * F], BF, kind="ExternalInput")
    w2 = nc.dram_tensor("w2", [P, 2 * F], BF, kind="ExternalInput")
    recip1 = nc.dram_tensor("recip1", [P, F], F32, kind="ExternalInput")
    recip2 = nc.dram_tensor("recip2", [P, F], F32, kind="ExternalInput")
    bb1 = nc.dram_tensor("bb1", [P, F], F32, kind="ExternalInput")
    bb2 = nc.dram_tensor("bb2", [P, F], F32, kind="ExternalInput")
    identB = nc.dram_tensor("identB", [P, P], BF, kind="ExternalInput")
    identB8 = nc.dram_tensor("identB8", [P, P], F8, kind="ExternalInput")
    identF = nc.dram_tensor("identF", [P, P], F32, kind="ExternalInput")
    out = nc.dram_tensor("out", [SHARD, F], F32, kind="ExternalOutput")

    eq = mybir.AluOpType.is_equal
    mul = mybir.AluOpType.mult
    AF = mybir.ActivationFunctionType
    AX = mybir.AxisListType.X

    with TileContext(nc) as tc:
        with (
            tc.tile_pool(name="const", bufs=1) as cpool,
            tc.tile_pool(name="big", bufs=1) as bigp,
            tc.tile_pool(name="work", bufs=1) as wp,
            tc.tile_pool(name="psum", bufs=1, space="PSUM") as pp,
            tc.tile_pool(name="dram", bufs=1, space="DRAM") as dp,
        ):
            def load_const(t, shape, dt):
                s = cpool.tile(shape, dt, name=t.name + "_sb")
                nc.sync.dma_start(out=s[:], in_=t[:])
                return s
            w1_sb = load_const(w1, [P, 2 * F], BF)
            w2_sb = load_const(w2, [P, 2 * F], BF)
            recip1_sb = load_const(recip1, [P, F], F32)
            recip2_sb = load_const(recip2, [P, F], F32)
            bb1_sb = load_const(bb1, [P, F], F32)
            bb2_sb = load_const(bb2, [P, F], F32)
            identB_sb = load_const(identB, [P, P], BF)
            identB8_sb = load_const(identB8, [P, P], F8)
            identF_sb = load_const(identF, [P, P], F32)
            xTs_sb = bigp.tile([P, SHARD], BF, name="xTs_sb")
            nc.sync.dma_start(out=xTs_sb[:], in_=xTs[:])
            gidx_sb = bigp.tile([P, CALLS * (NI // 16)], I16, name="gidx_sb")
            nc.sync.dma_start(out=gidx_sb[:], in_=gidx[:])

            lneps_sb = cpool.tile([P, 1], F32, name="lneps_sb")
            nc.vector.memset(lneps_sb[:], float(np.log(1e-16)))
            xr1_sb = bigp.tile([P, SHARD], BF, name="xr1_sb")
            xr2_sb = bigp.tile([P, SHARD], BF, name="xr2_sb")
            hT_sb = bigp.tile([P, SHARD], BF, name="hT_sb")
            hacc = bigp.tile([P, NBLK * RW], F32, name="hacc")
            stage = bigp.tile([P, SHARD], BF, name="stage")      # xl (bf16)
            stage_o = bigp.tile([P, SHARD], F32, name="stage_o")  # epilogue f32

            xl1sh = dp.tile([SHARD, F], BF, name="xl1sh")
            xl1full = dp.tile([NPAD, F], BF, name="xl1full", addr_space="Shared")
            xl2sh = dp.tile([SHARD, F], BF, name="xl2sh")
            xl2full = dp.tile([NPAD, F], BF, name="xl2full", addr_space="Shared")

            dma_sem = nc.alloc_semaphore("gat_dma")

            def node_phase(src_sb, w_sb, xr_dst):
                for j in range(NBLK):
                    mm = pp.tile([P, 8, P], F32, tag="txr", bufs=2, name=f"mm{j}")
                    mf = mm[:].rearrange("p c f -> p (c f)")
                    nc.tensor.matmul(out=mf[:, 0:2 * F],
                                     lhsT=src_sb[:, j * P:(j + 1) * P],
                                     rhs=w_sb[:], start=True, stop=True)
                    nc.scalar.activation(out=stage[:, j * F:(j + 1) * F],
                                         in_=mf[:, 0:F], func=AF.Copy)
                    nc.scalar.activation(out=xr_dst[:, j * P:(j + 1) * P],
                                         in_=mf[:, F:2 * F], func=AF.Copy)

            def dma_stage_to(dram_tile):
                o = dram_tile[:].rearrange("(b p) f -> p b f", p=P)
                i = stage[:].rearrange("p (b f) -> p b f", f=F)
                nc.sync.dma_start(out=o, in_=i)

            def edge_pass(layer, table, xr_sb, ks):
                HN = cfg.H1 if layer == 1 else 1
                CW = F // HN
                RWB = F + 3 * HN
                nc.vector.memset(hacc[:], 0.0)
                bp = None
                for g in range(CALLS):
                    cb0 = g * GPC
                    hf = int(chunk_half[cb0])
                    tab = table[:][0:HALF, :] if hf == 0 else table[:][HALF:NPAD, :]
                    xg = wp.tile([P, GPC, F], BF, tag="xg", bufs=6,
                                 name=f"xg{layer}_{g}")
                    if USE_PREP:
                        nc.gpsimd.dma_gather(
                            out_ap=xg[:], in_ap=tab,
                            idxs_ap=gidx_sb[:, g * (NI // 16):(g + 1) * (NI // 16)],
                            num_idxs=NI, num_idxs_reg=NI, elem_size=F,
                            prepare_only=True, sem=dma_sem, queue_num=0)
                        nc.gpsimd.trigger_dma(count=None)
                    else:
                        nc.gpsimd.dma_gather(
                            out_ap=xg[:], in_ap=tab,
                            idxs_ap=gidx_sb[:, g * (NI // 16):(g + 1) * (NI // 16)],
                            num_idxs=NI, num_idxs_reg=NI, elem_size=F,
                            queue_num=g % 4)
                    for sub in range(GPC // CPC):
                        cb = cb0 + sub * CPC
                        # one-hot QT[e, n] (lhsT for scatter) streamed from host
                        qt = wp.tile([P, CPC, P], F8, tag="qt", bufs=8,
                                     name=f"qt{layer}_{cb}")
                        nc.sync.dma_start(
                            out=qt[:],
                            in_=qts[:, cb * P:(cb + CPC) * P].rearrange(
                                "p (c f) -> p c f", f=P))
                        # Q[n, e] one-hot = PE transpose of QT; copy to SBUF on ACT
                        # fp8 transpose writes with element step 2 (16-bit
                        # granularity); tile holds 2*CPC*P fp8 = 2048 B
                        trp = pp.tile([P, 2 * CPC * P], F8, tag="trp", bufs=2,
                                      name=f"trp{layer}_{cb}")
                        tpb = trp[:]
                        for c in range(CPC):
                            nc.tensor.transpose(
                                out=_ap(tpb, [tpb.ap[0], [2, P]], 2 * c * P),
                                in_=qt[:, c, :], identity=identB8_sb[:])
                        q = wp.tile([P, CPC, P], F8, tag="q", bufs=8,
                                    name=f"q{layer}_{cb}")
                        nc.scalar.activation(
                            out=q[:],
                            in_=_ap(tpb, [tpb.ap[0], [2 * P, CPC], [2, P]]),
                            func=AF.Copy)
                        # t~ = Q.T @ xr (+ xg via identity matmul), PSUM f32
                        txr = pp.tile([P, CPC, P], F32, tag="txr", bufs=2,
                                      name=f"txr{layer}_{cb}")
                        for c in range(CPC):
                            blk = int(chunk_block[cb + c])
                            nc.tensor.matmul(
                                out=txr[:, c, :], lhsT=q[:, c, :],
                                rhs=xr_sb[:, blk * P:(blk + 1) * P],
                                start=True, stop=False)
                            nc.tensor.matmul(
                                out=txr[:, c, :], lhsT=identB_sb[:],
                                rhs=xg[:, sub * CPC + c, :],
                                start=False, stop=True)
                        # att_c*leaky(t_c): Prelu(x;.2) pos block,
                        # Prelu(.2x;5)=min(x,.2x) neg block, per head
                        lr = wp.tile([P, CPC, F], BF, tag="lr", bufs=4,
                                     name=f"lr{layer}_{cb}")
                        tb = txr[:]
                        lb = lr[:]
                        for h in range(HN):
                            kh = ks[h]
                            if kh > 0:
                                ap_i = bass.AP(tb.tensor, tb.offset + h * CW,
                                               [list(tb.ap[0]), [F, CPC], [1, kh]])
                                ap_o = bass.AP(lb.tensor, lb.offset + h * CW,
                                               [list(lb.ap[0]), [F, CPC], [1, kh]])
                                nc.scalar.activation(out=ap_o, in_=ap_i,
                                                     func=AF.Prelu, alpha=0.2)
                            if kh < CW:
                                off = h * CW + kh
                                ap_i = bass.AP(tb.tensor, tb.offset + off,
                                               [list(tb.ap[0]), [F, CPC],
                                                [1, CW - kh]])
                                ap_o = bass.AP(lb.tensor, lb.offset + off,
                                               [list(lb.ap[0]), [F, CPC],
                                                [1, CW - kh]])
                                nc.scalar.activation(out=ap_o, in_=ap_i,
                                                     func=AF.Prelu, alpha=5.0,
                                                     scale=0.2)
                        # alpha = per-head sum
                        al = wp.tile([P, CPC * HN], F32, tag="al", bufs=6,
                                     name=f"al{layer}_{cb}")
                        nc.vector.reduce_sum(
                            out=al[:],
                            in_=lr[:].rearrange("p c (h s) -> p c h s", s=CW),
                            axis=AX)
                        # p = exp(alpha) (bf16)
                        pe = wp.tile([P, CPC * HN], BF, tag="pe", bufs=6,
                                     name=f"pe{layer}_{cb}")
                        nc.scalar.activation(out=pe[:], in_=al[:], func=AF.Exp)
                        # thi = bf16(al + 8): snaps al to the exact bf16 grid
                        thi = wp.tile([P, CPC * HN], BF, tag="thi", bufs=6,
                                      name=f"thi{layer}_{cb}")
                        nc.scalar.activation(out=thi[:], in_=al[:], func=AF.Copy,
                                             bias=8.0)
                        # rhs = [p*xg | p | hi | lo]
                        rhs = wp.tile([P, CPC, RWB], BF, tag="rhs", bufs=6,
                                      name=f"rhs{layer}_{cb}")
                        rb = rhs[:]
                        xb = xg[:, sub * CPC:(sub + 1) * CPC, :]
                        pb = pe[:]
                        nc.vector.tensor_tensor(
                            out=_ap(rb, [rb.ap[0], [RWB, CPC], [CW, HN], [1, CW]]),
                            in0=_ap(xb, [xb.ap[0], [F, CPC], [CW, HN], [1, CW]]),
                            in1=_ap(pb, [pb.ap[0], [HN, CPC], [1, HN], [0, CW]]),
                            op=mul)
                        pc_out = bass.AP(rb.tensor, rb.offset + F,
                                         [list(rb.ap[0]), [RWB, CPC], [1, HN]])
                        nc.scalar.activation(
                            out=pc_out, in_=pb.rearrange("p (c h) -> p c h", h=HN),
                            func=AF.Copy)
                        hi_out = bass.AP(rb.tensor, rb.offset + F + HN,
                                         [list(rb.ap[0]), [RWB, CPC], [1, HN]])
                        nc.scalar.activation(
                            out=hi_out, in_=thi[:].rearrange(
                                "p (c h) -> p c h", h=HN),
                            func=AF.Copy, bias=-8.0)
                        lo_out = bass.AP(rb.tensor, rb.offset + F + 2 * HN,
                                         [list(rb.ap[0]), [RWB, CPC], [1, HN]])
                        nc.vector.tensor_tensor(
                            out=lo_out,
                            in0=al[:].rearrange("p (c h) -> p c h", h=HN),
                            in1=hi_out, op=mybir.AluOpType.subtract)
                        # scatter matmuls, PSUM-accumulated per block segment
                        for c in range(CPC):
                            ci = cb + c
                            blk = int(chunk_block[ci])
                            seg_start = ci == 0 or chunk_block[ci - 1] != blk
                            seg_end = ci == CH - 1 or chunk_block[ci + 1] != blk
                            if seg_start:
                                bp = pp.tile([P, RWB], F32, tag="bp", bufs=2,
                                             name=f"bp{layer}_{ci}")
                            nc.tensor.matmul(
                                out=bp[:], lhsT=qt[:, c, :], rhs=rhs[:, c, :],
                                start=seg_start, stop=seg_end)
                            if seg_end:
                                nc.vector.tensor_add(
                                    out=hacc[:, blk * RW:blk * RW + RWB],
                                    in0=hacc[:, blk * RW:blk * RW + RWB],
                                    in1=bp[:])

            def epilogue(layer, recip_sb, bb_sb):
                HN = cfg.H1 if layer == 1 else 1
                CW = F // HN
                NB = NBLK
                hb = hacc[:]
                # batched across all blocks: sa = hi + lo, eps, den, rec
                sa = wp.tile([P, NB, HN], F32, tag="sa", bufs=1,
                             name=f"sa{layer}")
                nc.vector.tensor_add(
                    out=sa[:],
                    in0=_ap(hb, [hb.ap[0], [RW, NB], [1, HN]], F + HN),
                    in1=_ap(hb, [hb.ap[0], [RW, NB], [1, HN]], F + 2 * HN))
                eps = wp.tile([P, NB, HN], F32, tag="eps", bufs=1,
                              name=f"eps{layer}")
                nc.scalar.activation(out=eps[:], in_=sa[:], func=AF.Exp,
                                     bias=lneps_sb[:, 0:1])
                den = wp.tile([P, NB, HN], F32, tag="den", bufs=1,
                              name=f"den{layer}")
                nc.vector.tensor_add(
                    out=den[:], in0=_ap(hb, [hb.ap[0], [RW, NB], [1, HN]], F),
                    in1=eps[:])
                rec = wp.tile([P, NB, HN], F32, tag="rec", bufs=1,
                              name=f"rec{layer}")
                nc.vector.reciprocal(out=rec[:], in_=den[:])
                # sc[n, b, c] = rec[n, b, head(c)] * recip_att[c]
                sc = wp.tile([P, NB * F], F32, tag="sc", bufs=1,
                             name=f"sc{layer}")
                scb = sc[:]
                rcb = rec[:]
                rpb = recip_sb[:]
                nc.vector.tensor_tensor(
                    out=_ap(scb, [scb.ap[0], [F, NB], [CW, HN], [1, CW]]),
                    in0=_ap(rcb, [rcb.ap[0], [HN, NB], [1, HN], [0, CW]]),
                    in1=_ap(rpb, [rpb.ap[0], [0, NB], [CW, HN], [1, CW]]),
                    op=mul)
                # y = msg * sc + bias
                so2 = stage_o[:].rearrange("p (b f) -> p b f", f=F)
                nc.vector.tensor_tensor(
                    out=so2, in0=_ap(hb, [hb.ap[0], [RW, NB], [1, F]]),
                    in1=sc[:].rearrange("p (b f) -> p b f", f=F), op=mul)
                bbb = bb_sb[:]
                nc.vector.tensor_tensor(
                    out=so2, in0=so2,
                    in1=_ap(bbb, [bbb.ap[0], [0, NB], [1, F]]),
                    op=mybir.AluOpType.add)
                sob = stage_o[:]
                # elu(y) = relu(y) + exp(-relu(-y)) - 1
                # tmp reuse: stage (xl staging, dead) and xTs (dead after L1)
                nc.scalar.activation(out=stage[:], in_=sob, func=AF.Relu,
                                     scale=-1.0)
                nc.scalar.activation(out=xTs_sb[:], in_=stage[:], func=AF.Exp,
                                     scale=-1.0)
                nc.vector.tensor_scalar_max(out=sob, in0=sob, scalar1=0.0)
                nc.vector.tensor_add(out=sob, in0=sob, in1=xTs_sb[:])
                nc.vector.tensor_scalar_add(out=sob, in0=sob, scalar1=-1.0)
                if layer == 1:
                    for b in range(NBLK):
                        trh = pp.tile([P, 512], F32, tag="trp", bufs=2,
                                      name=f"trh{b}")
                        nc.tensor.transpose(out=trh[:, 0:P],
                                            in_=stage_o[:, b * F:(b + 1) * F],
                                            identity=identF_sb[:])
                        nc.scalar.activation(out=hT_sb[:, b * P:(b + 1) * P],
                                             in_=trh[:, 0:P], func=AF.Copy)

            # ---- layer 1 ----
            node_phase(xTs_sb, w1_sb, xr1_sb)
            dma_stage_to(xl1sh)
            nc.gpsimd.collective_compute(
                "AllGather", mybir.AluOpType.bypass,
                replica_groups=[list(range(8))],
                ins=[xl1sh[:]], outs=[xl1full[:]])
            edge_pass(1, xl1full, xr1_sb, k1)
            epilogue(1, recip1_sb, bb1_sb)
            # ---- layer 2 ----
            node_phase(hT_sb, w2_sb, xr2_sb)
            dma_stage_to(xl2sh)
            nc.gpsimd.collective_compute(
                "AllGather", mybir.AluOpType.bypass,
                replica_groups=[list(range(8))],
                ins=[xl2sh[:]], outs=[xl2full[:]])
            edge_pass(2, xl2full, xr2_sb, k2)
            epilogue(2, recip2_sb, bb2_sb)
            oo = out[:].rearrange("(b p) f -> p b f", p=P)
            ii = stage_o[:].rearrange("p (b f) -> p b f", f=F)
            nc.sync.dma_start(out=oo, in_=ii)

    nc.compile()
    return nc


# ---------------------------------------------------------------------------
# public entry point
# ---------------------------------------------------------------------------
_CACHE = {}
LAST_RESULTS = None


def _trace_enabled():
    import os
    return os.environ.get("GAT_TRACE", "") == "1"


def _install_trace_shim():
    """antenv.axon_hooks is absent in this image; recreate it so trace=True
    can capture NTFF profiles through the axon PJRT plugin."""
    import sys, types
    if "antenv.axon_hooks" in sys.modules:
        return
    try:
        mod = types.ModuleType("antenv.axon_hooks")
        mod._hook = None
        mod.set_axon_ntff_profile_hook = lambda h: setattr(mod, "_hook", h)
        mod.get_axon_ntff_profile_hook = lambda: mod._hook
        sys.modules["antenv.axon_hooks"] = mod
        import antenv
        antenv.axon_hooks = mod
        from trn_agent_boot.trn_boot import _ntff_profile_via_ctypes
        mod._hook = _ntff_profile_via_ctypes("/opt/axon/libaxon_pjrt.so")
        import concourse.bass_utils as bu
        bu.upload_artifacts = lambda tmpdir: str(tmpdir)
    except Exception:
        pass


def kernel(x, edge_index, Wl1, Wr1, att1, b1, Wl2, Wr2, att2, b2):
    global LAST_RESULTS
    from concourse.bass_utils import run_bass_kernel_spmd

    trace = _trace_enabled()
    if trace:
        _install_trace_shim()

    x = np.asarray(x, np.float32)
    edge_index = np.asarray(edge_index)
    N, E = x.shape[0], edge_index.shape[1]
    cfg = Cfg(N, E, nblk=49)

    per_core, struct = host_prep(cfg, x, edge_index)
    consts, meta = host_consts(cfg, Wl1, Wr1, att1, b1, Wl2, Wr2, att2, b2)

    key = (N, E, x.shape[1], struct["S_A"], struct["S_B"],
           meta["k1"], meta["k2"])
    if key not in _CACHE:
        _CACHE[key] = build_program(cfg, struct, meta["k1"], meta["k2"])
    nc = _CACHE[key]

    in_maps = []
    for k in range(8):
        m = dict(per_core[k])
        m.update(consts)
        in_maps.append(m)
    res = run_bass_kernel_spmd(nc, in_maps, core_ids=list(range(8)), trace=trace)
    LAST_RESULTS = res
    outs = [np.asarray(res.results[k]["out"]) for k in range(8)]
    full = np.concatenate(outs, axis=0)[:N].astype(np.float32)
    unperm = np.empty_like(full)
    unperm[:, meta["perm2"]] = full
    return unperm
